# revision 9
# baseline (speedup 1.0000x reference)
"""Trainium2 Bass kernel for nn_CosSimRouter_learn_49778670960796.

Host: cosine-similarity scoring / sort / gather (tiny, shape-determining).
Device (8 NeuronCores, tensor-parallel over heads/hidden):
  3x MHA + FFN + logits; fp32 storage, float32r matmuls; AllReduce after
  out-proj / FFN2 (Megatron-style TP). Activations feature-major [E, L].
Host: top-k + final gather (exact rows of the input).
"""

import numpy as np

E = 4096
H = 16
HID = 8192
GAMMA = 0.2
TEMP = 0.05
EXPAND = 0.7
NCORES = 8
ET = E // 128  # 32 feature tiles
DH = E // H  # 256
HL = H // NCORES  # 2 heads per core
DLOC = HL * DH  # 512 local head dims
FLOC = HID // NCORES  # 1024 local ffn hidden

_CACHE = {}


# ----------------------------------------------------------------------------
# host-side reference math (numpy, fp32) for the scoring stage + fallback
# ----------------------------------------------------------------------------

def _score_partition(vision_feature, text_embed, attention_mask):
    vf = vision_feature.astype(np.float32)
    te = text_embed.astype(np.float32)
    vn = vf / np.maximum(np.linalg.norm(vf, axis=-1, keepdims=True), 1e-8)
    tn = te / np.maximum(np.linalg.norm(te, axis=-1, keepdims=True), 1e-8)
    cs = vn @ tn.T
    cs = np.where(attention_mask[None, :], cs, np.float32(0.0))
    m = cs.max(axis=-1) / np.float32(TEMP)
    e = np.exp(m - m.max())
    scores = e / e.sum()
    order = np.argsort(-scores, kind="stable")
    cum = np.cumsum(scores[order])
    t = int((cum <= GAMMA).sum())
    return t, order[:t], order[t:]


def _ln_np(x):
    m = x.mean(-1, keepdims=True)
    v = ((x - m) ** 2).mean(-1, keepdims=True)
    return (x - m) / np.sqrt(v + 1e-5)


def _gelu_np(x):
    import math

    erf = np.frompyfunc(math.erf, 1, 1)
    return (x * 0.5 * (1.0 + erf(x / math.sqrt(2.0)).astype(np.float64))
            ).astype(x.dtype)


def _mha_np(q_in, kv_in, Wqkv, bqkv, Wo, bo):
    dh = E // H
    Wq, Wk, Wv = np.split(Wqkv, 3, axis=0)
    bq, bk, bv = np.split(bqkv, 3)
    q = (q_in @ Wq.T + bq).reshape(-1, H, dh)
    k = (kv_in @ Wk.T + bk).reshape(-1, H, dh)
    v = (kv_in @ Wv.T + bv).reshape(-1, H, dh)
    att = np.einsum("qhd,khd->hqk", q, k) / np.float32(np.sqrt(dh))
    att = att - att.max(-1, keepdims=True)
    att = np.exp(att)
    att /= att.sum(-1, keepdims=True)
    o = np.einsum("hqk,khd->qhd", att.astype(np.float32), v).reshape(-1, E)
    return o @ Wo.T + bo


def _reference_np(vision_feature, text_embed, attention_mask,
                  Wqkv1, bqkv1, Wo1, bo1, Wqkv2, bqkv2, Wo2, bo2,
                  Wqkvc, bqkvc, Woc, boc, Wf1, bf1, Wf2, bf2, Ws, bs):
    t, sel_idx, rem_idx = _score_partition(vision_feature, text_embed,
                                           attention_mask)
    sel = vision_feature[sel_idx]
    rem = vision_feature[rem_idx]
    cat = np.concatenate([sel, text_embed], axis=0)
    x = _ln_np(_mha_np(cat, cat, Wqkv1, bqkv1, Wo1, bo1) + cat)
    r = _ln_np(_mha_np(rem, rem, Wqkv2, bqkv2, Wo2, bo2) + rem)
    x = _ln_np(_mha_np(r, x, Wqkvc, bqkvc, Woc, boc) + r)
    ffn = _gelu_np(x @ Wf1.T + bf1) @ Wf2.T + bf2
    x = _ln_np(x + ffn)
    logits = (x @ Ws.T + bs).squeeze(-1)
    es = 1.0 / (1.0 + np.exp(-logits))
    k = int(t * EXPAND)
    ei = np.argsort(-es, kind="stable")[:k]
    final = np.sort(np.concatenate([sel_idx, rem_idx[ei]]))
    return vision_feature[final]


# ----------------------------------------------------------------------------
# device program
# ----------------------------------------------------------------------------

def _pad128(n):
    return ((n + 127) // 128) * 128


def _build_device(ncat_real, nrem_real, debug=False):
    import concourse.bacc as bacc
    import concourse.mybir as mybir
    import concourse.tile as tile

    dt = mybir.dt
    F32 = dt.float32
    F32R = dt.float32r
    AF = mybir.ActivationFunctionType
    ALU = mybir.AluOpType

    ncat = _pad128(ncat_real)
    nrem = _pad128(nrem_real)
    JC = ncat // 128  # kv tiles for cat (2)
    JR = nrem // 128  # kv tiles for rem (4)

    nc = bacc.Bacc("TRN2", target_bir_lowering=False, debug=False,
                   num_devices=NCORES)

    # ---------------- DRAM I/O ----------------
    catT_d = nc.dram_tensor("catT", [E, ncat], F32, kind="ExternalInput")
    remT_d = nc.dram_tensor("remT", [E, nrem], F32, kind="ExternalInput")
    wd = {}
    for l in ("1", "2", "c"):
        for p in ("q", "k", "v"):
            wd[p + l] = nc.dram_tensor(f"w{p}{l}", [E, DLOC], F32,
                                       kind="ExternalInput")
        wd["o" + l] = nc.dram_tensor(f"wo{l}", [DLOC, E], F32,
                                     kind="ExternalInput")
    wd["f1"] = nc.dram_tensor("wf1", [E, FLOC], F32, kind="ExternalInput")
    wd["f2"] = nc.dram_tensor("wf2", [FLOC, E], F32, kind="ExternalInput")
    wd["s"] = nc.dram_tensor("ws", [128, ET], F32, kind="ExternalInput")
    masks_d = nc.dram_tensor("masks", [128, 2], F32, kind="ExternalInput")
    logits_d = nc.dram_tensor("logits", [1, nrem], F32, kind="ExternalOutput")
    dbg = {}
    if debug:
        for nm, L in (("dbg_x1", ncat), ("dbg_r", nrem), ("dbg_x2", nrem),
                      ("dbg_x3", nrem)):
            dbg[nm] = nc.dram_tensor(nm, [E, L], F32, kind="ExternalOutput")

    replica = [list(range(NCORES))]

    with tile.TileContext(nc, num_cores=NCORES) as tc:
        with (
            tc.tile_pool(name="acts", bufs=1) as acts,
            tc.tile_pool(name="psum", bufs=1, space="PSUM") as psum,
            tc.tile_pool(name="dram", bufs=1, space="DRAM") as dram,
        ):
            # ---- constants / packed stat tiles ----
            ones_col = acts.tile([128, 1], F32R, name="ones_col",
                                 tag="ones_col")
            nc.vector.memset(ones_col[:].bitcast(F32), 1.0)
            ones_row = acts.tile([1, 128], F32R, name="ones_row",
                                 tag="ones_row")
            nc.vector.memset(ones_row[:].bitcast(F32), 1.0)
            masks = acts.tile([128, 2], F32R, name="masks", tag="masks")
            nc.gpsimd.dma_start(masks[:], masks_d.ap())

            def pp(name, L):
                return psum.tile([128, L], F32, name=name, tag="pp", bufs=8)

            def pstat(name, L):
                return psum.tile([1, L], F32, name=name, tag="pp", bufs=8)

            def wtile(name, cols):
                return acts.tile([128, cols], F32R, name=name, tag="wt",
                                 bufs=6, padded_shape=[128, 1024])

            # ---------------- building blocks ----------------
            def load_xT(name, dram_t, L, tagbase):
                ts = []
                for k in range(ET):
                    xt = acts.tile([128, L], F32R, name=f"{name}_{k}",
                                   tag=f"{tagbase}_{k}")
                    nc.gpsimd.dma_start(xt[:],
                                        dram_t.ap()[128 * k:128 * (k + 1), :])
                    ts.append(xt)
                return ts

            def proj_fm(tagbase, w_dram, x_tiles, L, outtag):
                """q/k fm projection -> 4 tiles [128, L] (f32r)."""
                ps = [pp(f"ps_{tagbase}_{m}", L) for m in range(4)]
                outs = []
                for k in range(ET):
                    wt = wtile(f"w_{tagbase}_{k}", DLOC)
                    nc.gpsimd.dma_start(
                        wt[:], w_dram.ap()[128 * k:128 * (k + 1), :])
                    for m in range(4):
                        nc.tensor.matmul(ps[m][:],
                                         wt[:, 128 * m:128 * (m + 1)],
                                         x_tiles[k][:],
                                         start=(k == 0), stop=(k == ET - 1))
                for m in range(4):
                    o = acts.tile([128, L], F32R, name=f"{tagbase}_{m}",
                                  tag=f"{outtag}_{m}")
                    nc.vector.tensor_copy(o[:], ps[m][:])
                    outs.append(o)
                return outs

            def proj_tm(tagbase, w_dram, x_tiles, L):
                """v tm projection -> L//128 tiles [128, DLOC] (f32r)."""
                jt = L // 128
                ps = [pp(f"ps_{tagbase}_{j}", DLOC) for j in range(jt)]
                outs = []
                for k in range(ET):
                    wt = wtile(f"w_{tagbase}_{k}", DLOC)
                    nc.gpsimd.dma_start(
                        wt[:], w_dram.ap()[128 * k:128 * (k + 1), :])
                    for j in range(jt):
                        nc.tensor.matmul(ps[j][:],
                                         x_tiles[k][:, 128 * j:128 * (j + 1)],
                                         wt[:],
                                         start=(k == 0), stop=(k == ET - 1))
                for j in range(jt):
                    o = acts.tile([128, DLOC], F32R, name=f"{tagbase}_{j}",
                                  tag=f"v_{j}")
                    nc.vector.tensor_copy(o[:], ps[j][:])
                    outs.append(o)
                return outs

            def attention(tag, qT, kT, vT, Lq, Lkv, kv_valid, mask_idx):
                jt = Lkv // 128
                oT = []
                for h in range(HL):
                    exps = []
                    for j in range(jt):
                        p = pp(f"ps_s_{tag}_{h}_{j}", Lq)
                        for c in range(2):
                            nc.tensor.matmul(
                                p[:],
                                kT[2 * h + c][:, 128 * j:128 * (j + 1)],
                                qT[2 * h + c][:],
                                start=(c == 0), stop=(c == 1))
                        e = acts.tile([128, Lq], F32R,
                                      name=f"es_{tag}_{h}_{j}",
                                      tag=f"expS_{j}")
                        nc.scalar.activation(e[:], p[:], AF.Exp,
                                             scale=float(1.0 / np.sqrt(DH)))
                        exps.append(e)
                    dsum = pstat(f"ps_d_{tag}_{h}", Lq)
                    for j in range(jt):
                        if j == jt - 1 and kv_valid < Lkv:
                            col = masks[:, mask_idx:mask_idx + 1]
                        else:
                            col = ones_col[:]
                        nc.tensor.matmul(dsum[:], col, exps[j][:],
                                         start=(j == 0), stop=(j == jt - 1))
                    den = acts.tile([1, Lq], F32, name=f"den_{tag}_{h}",
                                    tag="aden")
                    rec = acts.tile([1, Lq], F32, name=f"rec_{tag}_{h}",
                                    tag="arec")
                    nc.vector.tensor_copy(den[:], dsum[:])
                    nc.vector.reciprocal(rec[:], den[:])
                    nc.vector.tensor_tensor(den[:], den[:], rec[:], ALU.mult)
                    nc.vector.tensor_scalar(den[:], den[:], -1.0, 2.0,
                                            ALU.mult, ALU.add)
                    rec2 = acts.tile([1, Lq], F32R, name=f"rec2_{tag}_{h}",
                                     tag="rec2")
                    nc.vector.tensor_tensor(rec2[:], rec[:], den[:], ALU.mult)
                    rrep_p = pp(f"ps_rr_{tag}_{h}", Lq)
                    nc.tensor.matmul(rrep_p[:], ones_row[:], rec2[:],
                                     start=True, stop=True)
                    rrep = acts.tile([128, Lq], F32, name=f"rr_{tag}_{h}",
                                     tag="rrep")
                    nc.scalar.copy(rrep[:], rrep_p[:])
                    for c in range(2):
                        po = pp(f"ps_o_{tag}_{h}_{c}", Lq)
                        for j in range(jt):
                            nc.tensor.matmul(
                                po[:],
                                vT[j][:, 256 * h + 128 * c:
                                      256 * h + 128 * (c + 1)],
                                exps[j][:],
                                start=(j == 0), stop=(j == jt - 1))
                        o = acts.tile([128, Lq], F32R,
                                      name=f"oT_{tag}_{h}_{c}",
                                      tag=f"oT_{2 * h + c}")
                        nc.vector.tensor_tensor(o[:], po[:], rrep[:],
                                                ALU.mult)
                        oT.append(o)
                return oT

            def out_proj_to_dram(tag, oT, w_dram, ar_in, Lq):
                for quarter in range(4):
                    wo_t = []
                    for k in range(4):
                        wt = wtile(f"wo_{tag}_{quarter}_{k}", 1024)
                        nc.gpsimd.dma_start(
                            wt[:],
                            w_dram.ap()[128 * k:128 * (k + 1),
                                        1024 * quarter:1024 * (quarter + 1)])
                        wo_t.append(wt)
                    ps = []
                    for mm in range(8):
                        m = 8 * quarter + mm
                        ps.append(pp(f"ps_op_{tag}_{m}", Lq))
                    for k in range(4):
                        for mm in range(8):
                            nc.tensor.matmul(
                                ps[mm][:],
                                wo_t[k][:, 128 * mm:128 * (mm + 1)],
                                oT[k][:],
                                start=(k == 0), stop=(k == 3))
                    for mm in range(8):
                        m = 8 * quarter + mm
                        st = acts.tile([128, Lq], F32,
                                       name=f"st_{tag}_{m}", tag="stage",
                                       bufs=3)
                        nc.vector.tensor_copy(st[:], ps[mm][:])
                        nc.sync.dma_start(
                            ar_in[128 * m:128 * (m + 1), :], st[:])

            def do_allreduce(tag, ar_in, ar_out):
                nc.gpsimd.collective_compute(
                    "AllReduce", ALU.add, replica_groups=replica,
                    ins=[ar_in.opt()], outs=[ar_out.opt()])

            def residual_ln(tag, ar_out, res_tiles, L, valid=None,
                            dump=None):
                """In-place: res_tiles[k] <- LN(ar_out + res_tiles)[k]."""
                # xsum (in-place into res slot)
                for k in range(ET):
                    b = acts.tile([128, L], F32, name=f"arb_{tag}_{k}",
                                  tag="arb", bufs=4)
                    nc.sync.dma_start(b[:], ar_out[128 * k:128 * (k + 1), :])
                    nc.vector.tensor_tensor(res_tiles[k][:], b[:],
                                            res_tiles[k][:], ALU.add)
                s1p = pstat(f"ps_s1_{tag}", L)
                s2p = pstat(f"ps_s2_{tag}", L)
                for k in range(ET):
                    nc.tensor.matmul(s1p[:], ones_col[:], res_tiles[k][:],
                                     start=(k == 0), stop=(k == ET - 1))
                for k in range(ET):
                    sq = acts.tile([128, L], F32R, name=f"sq_{tag}_{k}",
                                   tag="stage", bufs=3)
                    nc.scalar.square(sq[:], res_tiles[k][:])
                    nc.tensor.matmul(s2p[:], ones_col[:], sq[:],
                                     start=(k == 0), stop=(k == ET - 1))
                mean = acts.tile([1, L], F32, name=f"mean_{tag}",
                                 tag="lmean")
                var = acts.tile([1, L], F32, name=f"var_{tag}", tag="lvar")
                tmpa = acts.tile([1, L], F32, name=f"tmpa_{tag}", tag="ltmp")
                r0 = acts.tile([1, L], F32, name=f"r0_{tag}", tag="lr0")
                nc.scalar.mul(mean[:], s1p[:], 1.0 / E)
                nc.scalar.mul(var[:], s2p[:], 1.0 / E)
                nc.scalar.square(tmpa[:], mean[:])
                nc.vector.tensor_sub(var[:], var[:], tmpa[:])
                nc.vector.tensor_scalar_add(var[:], var[:], 1e-5)
                nc.scalar.sqrt(tmpa[:], var[:])
                nc.vector.reciprocal(r0[:], tmpa[:])
                nc.vector.tensor_tensor(tmpa[:], r0[:], r0[:], ALU.mult)
                nc.vector.tensor_tensor(tmpa[:], tmpa[:], var[:], ALU.mult)
                nc.vector.tensor_scalar(tmpa[:], tmpa[:], -0.5, 1.5, ALU.mult,
                                        ALU.add)
                rstd = acts.tile([1, L], F32R, name=f"rstd_{tag}", tag="rstd")
                nmr = acts.tile([1, L], F32R, name=f"nmr_{tag}", tag="nmr")
                nc.vector.tensor_tensor(rstd[:], r0[:], tmpa[:], ALU.mult)
                nc.vector.scalar_tensor_tensor(nmr[:], mean[:], -1.0, rstd[:],
                                               ALU.mult, ALU.mult)
                Apsum = pp(f"ps_A_{tag}", L)
                nc.tensor.matmul(Apsum[:], ones_row[:], rstd[:], start=True,
                                 stop=True)
                Bpsum = pp(f"ps_B_{tag}", L)
                nc.tensor.matmul(Bpsum[:], ones_row[:], nmr[:], start=True,
                                 stop=True)
                Asb = acts.tile([128, L], F32, name=f"A_{tag}", tag="Asb")
                nc.scalar.copy(Asb[:], Apsum[:])
                Bsb = acts.tile([128, L], F32, name=f"B_{tag}", tag="Bsb")
                nc.scalar.copy(Bsb[:], Bpsum[:])
                for k in range(ET):
                    nc.vector.tensor_tensor(res_tiles[k][:], res_tiles[k][:],
                                            Asb[:], ALU.mult)
                    nc.vector.tensor_tensor(res_tiles[k][:], res_tiles[k][:],
                                            Bsb[:], ALU.add)
                    if valid is not None and valid < L:
                        nc.vector.memset(
                            res_tiles[k][:, valid:L].bitcast(F32), 0.0)
                    if dump is not None:
                        nc.sync.dma_start(
                            dump.ap()[128 * k:128 * (k + 1), :],
                            res_tiles[k][:].bitcast(F32))
                return res_tiles

            # ================= program =================
            # fm512 family ("a_{k}"): remT -> r -> x2 -> x3 (in-place chain)
            # fm256 family ("b_{k}"): catT -> x1
            a_t = load_xT("remT", remT_d, nrem, "a")

            # ---- MHA2 (rem self-attention) ----
            q2 = proj_fm("q2", wd["q2"], a_t, nrem, "q")
            k2 = proj_fm("k2", wd["k2"], a_t, nrem, "k")
            v2 = proj_tm("v2", wd["v2"], a_t, nrem)
            o2 = attention("a2", q2, k2, v2, nrem, nrem, nrem_real, 1)
            arin2 = dram.tile([E, nrem], F32, name="arin2", tag="arin2")
            arout2 = dram.tile([E, nrem], F32, name="arout2", tag="arout2", addr_space="Shared")
            out_proj_to_dram("op2", o2, wd["o2"], arin2, nrem)
            do_allreduce("2", arin2, arout2)

            # ---- MHA1 (cat self-attention), overlaps AR2 ----
            b_t = load_xT("catT", catT_d, ncat, "b")
            q1 = proj_fm("q1", wd["q1"], b_t, ncat, "q")
            k1 = proj_fm("k1", wd["k1"], b_t, ncat, "k")
            v1 = proj_tm("v1", wd["v1"], b_t, ncat)
            o1 = attention("a1", q1, k1, v1, ncat, ncat, ncat_real, 0)
            arin1 = dram.tile([E, ncat], F32, name="arin1", tag="arin1")
            arout1 = dram.tile([E, ncat], F32, name="arout1", tag="arout1", addr_space="Shared")
            out_proj_to_dram("op1", o1, wd["o1"], arin1, ncat)
            do_allreduce("1", arin1, arout1)

            # ---- LN stages: r = LN(AR2 + rem); x1 = LN(AR1 + cat) ----
            r_t = residual_ln("r", arout2, a_t, nrem, dump=dbg.get("dbg_r"))
            x1_t = residual_ln("x1", arout1, b_t, ncat, valid=ncat_real,
                               dump=dbg.get("dbg_x1"))

            # ---- MHAc (q from r, kv from x1) ----
            qc = proj_fm("qc", wd["qc"], r_t, nrem, "q")
            kc = proj_fm("kc", wd["kc"], x1_t, ncat, "k")
            vc = proj_tm("vc", wd["vc"], x1_t, ncat)
            oc = attention("ac", qc, kc, vc, nrem, ncat, ncat_real, 0)
            arinc = dram.tile([E, nrem], F32, name="arinc", tag="arinc")
            aroutc = dram.tile([E, nrem], F32, name="aroutc", tag="aroutc", addr_space="Shared")
            out_proj_to_dram("opc", oc, wd["oc"], arinc, nrem)
            do_allreduce("c", arinc, aroutc)
            x2_t = residual_ln("x2", aroutc, r_t, nrem,
                               dump=dbg.get("dbg_x2"))

            # ---- FFN ----
            # f1: hT = gelu(Wf1_shard @ x2): 8 psums, single weight sweep
            ps_f1 = [pp(f"ps_f1_{m}", nrem) for m in range(8)]
            for k in range(ET):
                wt = wtile(f"w_f1_{k}", FLOC)
                nc.gpsimd.dma_start(
                    wt[:], wd["f1"].ap()[128 * k:128 * (k + 1), :])
                for m in range(8):
                    nc.tensor.matmul(ps_f1[m][:],
                                     wt[:, 128 * m:128 * (m + 1)],
                                     x2_t[k][:],
                                     start=(k == 0), stop=(k == ET - 1))
            hT = []
            for m in range(8):
                tg = f"v_{m}" if m < 4 else f"q_{m - 4}"
                h = acts.tile([128, nrem], F32R, name=f"hT_{m}", tag=tg)
                nc.scalar.activation(h[:], ps_f1[m][:], AF.Gelu)
                hT.append(h)
            # f2: quarters of output cols; psum group of 8 m-tiles per quarter
            arin4 = dram.tile([E, nrem], F32, name="arin4", tag="arin4")
            arout4 = dram.tile([E, nrem], F32, name="arout4", tag="arout4", addr_space="Shared")
            HK = FLOC // 128  # 8
            for quarter in range(4):
                ps = []
                for mm in range(8):
                    m = 8 * quarter + mm
                    ps.append(pp(f"ps_f2_{m}", nrem))
                for khalf in range(2):
                    wf_t = []
                    for kk in range(4):
                        k = 4 * khalf + kk
                        wt = wtile(f"w_f2_{quarter}_{k}", 1024)
                        nc.gpsimd.dma_start(
                            wt[:],
                            wd["f2"].ap()[128 * k:128 * (k + 1),
                                          1024 * quarter:1024 * (quarter + 1)])
                        wf_t.append(wt)
                    for kk in range(4):
                        k = 4 * khalf + kk
                        for mm in range(8):
                            nc.tensor.matmul(
                                ps[mm][:],
                                wf_t[kk][:, 128 * mm:128 * (mm + 1)],
                                hT[k][:],
                                start=(k == 0), stop=(k == HK - 1))
                for mm in range(8):
                    m = 8 * quarter + mm
                    st = acts.tile([128, nrem], F32, name=f"st_f2_{m}",
                                   tag="stage", bufs=3)
                    nc.vector.tensor_copy(st[:], ps[mm][:])
                    nc.sync.dma_start(arin4[128 * m:128 * (m + 1), :], st[:])
            do_allreduce("4", arin4, arout4)
            x3_t = residual_ln("x3", arout4, x2_t, nrem,
                               dump=dbg.get("dbg_x3"))

            # ---- logits ----
            ws_sb = acts.tile([128, ET], F32R, name="ws_sb", tag="ws_sb")
            nc.gpsimd.dma_start(ws_sb[:], wd["s"].ap())
            lp = pstat("ps_logit", nrem)
            for k in range(ET):
                nc.tensor.matmul(lp[:], ws_sb[:, k:k + 1], x3_t[k][:],
                                 start=(k == 0), stop=(k == ET - 1))
            lsb = acts.tile([1, nrem], F32, name="lsb", tag="lsb")
            nc.vector.tensor_copy(lsb[:], lp[:])
            nc.sync.dma_start(logits_d.ap(), lsb[:])

    nc.compile()
    return nc


# ----------------------------------------------------------------------------
# host orchestration
# ----------------------------------------------------------------------------

def _prep_in_maps(vision_feature, text_embed, sel_idx, rem_idx, ncat, nrem,
                  Wqkv1, Wo1, Wqkv2, Wo2, Wqkvc, Woc, Wf1, Wf2, Ws):
    f32 = np.float32
    sel = vision_feature[sel_idx]
    rem = vision_feature[rem_idx]
    cat = np.concatenate([sel, text_embed], axis=0)
    catT = np.zeros((E, ncat), f32)
    catT[:, :cat.shape[0]] = cat.T
    remT = np.zeros((E, nrem), f32)
    remT[:, :rem.shape[0]] = rem.T

    ncat_real = cat.shape[0]
    nrem_real = rem.shape[0]
    masks = np.zeros((128, 2), f32)
    masks[:ncat_real - 128 * (ncat // 128 - 1), 0] = 1.0
    masks[:nrem_real - 128 * (nrem // 128 - 1), 1] = 1.0

    in_maps = []
    for c in range(NCORES):
        hs = slice(DLOC * c, DLOC * (c + 1))
        fs = slice(FLOC * c, FLOC * (c + 1))
        m = {"catT": catT, "remT": remT, "masks": masks,
             "ws": np.ascontiguousarray(Ws[0].reshape(ET, 128).T)}
        for l, Wqkv, Wo in (("1", Wqkv1, Wo1), ("2", Wqkv2, Wo2),
                            ("c", Wqkvc, Woc)):
            Wq, Wk, Wv = Wqkv[:E], Wqkv[E:2 * E], Wqkv[2 * E:]
            m["wq" + l] = np.ascontiguousarray(Wq[hs].T)
            m["wk" + l] = np.ascontiguousarray(Wk[hs].T)
            m["wv" + l] = np.ascontiguousarray(Wv[hs].T)
            m["wo" + l] = np.ascontiguousarray(Wo[:, hs].T)
        m["wf1"] = np.ascontiguousarray(Wf1[fs].T)
        m["wf2"] = np.ascontiguousarray(Wf2[:, fs].T)
        in_maps.append(m)
    return in_maps


def run_device(in_maps, ncat_real, nrem_real, debug=False, trace=False):
    from concourse.bass_utils import run_bass_kernel_spmd

    key = (ncat_real, nrem_real, debug)
    if key not in _CACHE:
        _CACHE[key] = _build_device(ncat_real, nrem_real, debug=debug)
    nc = _CACHE[key]
    return run_bass_kernel_spmd(nc, in_maps, list(range(NCORES)), trace=trace)


def _kernel_impl(inputs, debug=False, trace=False):
    vision_feature = np.asarray(inputs["vision_feature"], np.float32)
    text_embed = np.asarray(inputs["text_embed"], np.float32)
    attention_mask = np.asarray(inputs["attention_mask"])

    biases_zero = all(
        not np.any(np.asarray(inputs[b]))
        for b in ("bqkv1", "bo1", "bqkv2", "bo2", "bqkvc", "boc",
                  "bf1", "bf2", "bs"))
    if (not bool(attention_mask.all())) or (not biases_zero):
        return _reference_np(**{k: np.asarray(v) for k, v in inputs.items()}), None

    t, sel_idx, rem_idx = _score_partition(vision_feature, text_embed,
                                           attention_mask)
    ncat_real = t + text_embed.shape[0]
    nrem_real = vision_feature.shape[0] - t
    kk = int(t * EXPAND)

    in_maps = _prep_in_maps(
        vision_feature, text_embed, sel_idx, rem_idx,
        _pad128(ncat_real), _pad128(nrem_real),
        np.asarray(inputs["Wqkv1"], np.float32),
        np.asarray(inputs["Wo1"], np.float32),
        np.asarray(inputs["Wqkv2"], np.float32),
        np.asarray(inputs["Wo2"], np.float32),
        np.asarray(inputs["Wqkvc"], np.float32),
        np.asarray(inputs["Woc"], np.float32),
        np.asarray(inputs["Wf1"], np.float32),
        np.asarray(inputs["Wf2"], np.float32),
        np.asarray(inputs["Ws"], np.float32))
    res = run_device(in_maps, ncat_real, nrem_real, debug=debug, trace=trace)
    logits = res.results[0]["logits"][0, :nrem_real]
    es = (1.0 / (1.0 + np.exp(-logits.astype(np.float32))))
    ei = np.argsort(-es, kind="stable")[:kk]
    final = np.sort(np.concatenate([sel_idx, rem_idx[ei]]))
    return vision_feature[final], res


def kernel(**inputs):
    out, _ = _kernel_impl(inputs)
    return out


# revision 11
# speedup vs baseline: 1.0702x; 1.0702x over previous
"""Trainium2 Bass kernel for nn_CosSimRouter_learn_49778670960796.

Host: cosine-similarity scoring / sort / gather (tiny, shape-determining).
Device (8 NeuronCores, tensor-parallel over heads/hidden):
  3x MHA + FFN + logits; fp32 storage, float32r matmuls; AllReduce after
  out-proj / FFN2 (Megatron-style TP). Activations feature-major [E, L].
Host: top-k + final gather (exact rows of the input).
"""

import numpy as np

E = 4096
H = 16
HID = 8192
GAMMA = 0.2
TEMP = 0.05
EXPAND = 0.7
NCORES = 8
ET = E // 128  # 32 feature tiles
DH = E // H  # 256
HL = H // NCORES  # 2 heads per core
DLOC = HL * DH  # 512 local head dims
FLOC = HID // NCORES  # 1024 local ffn hidden

_CACHE = {}


# ----------------------------------------------------------------------------
# host-side reference math (numpy, fp32) for the scoring stage + fallback
# ----------------------------------------------------------------------------

def _score_partition(vision_feature, text_embed, attention_mask):
    vf = vision_feature.astype(np.float32)
    te = text_embed.astype(np.float32)
    vn = vf / np.maximum(np.linalg.norm(vf, axis=-1, keepdims=True), 1e-8)
    tn = te / np.maximum(np.linalg.norm(te, axis=-1, keepdims=True), 1e-8)
    cs = vn @ tn.T
    cs = np.where(attention_mask[None, :], cs, np.float32(0.0))
    m = cs.max(axis=-1) / np.float32(TEMP)
    e = np.exp(m - m.max())
    scores = e / e.sum()
    order = np.argsort(-scores, kind="stable")
    cum = np.cumsum(scores[order])
    t = int((cum <= GAMMA).sum())
    return t, order[:t], order[t:]


def _ln_np(x):
    m = x.mean(-1, keepdims=True)
    v = ((x - m) ** 2).mean(-1, keepdims=True)
    return (x - m) / np.sqrt(v + 1e-5)


def _gelu_np(x):
    import math

    erf = np.frompyfunc(math.erf, 1, 1)
    return (x * 0.5 * (1.0 + erf(x / math.sqrt(2.0)).astype(np.float64))
            ).astype(x.dtype)


def _mha_np(q_in, kv_in, Wqkv, bqkv, Wo, bo):
    dh = E // H
    Wq, Wk, Wv = np.split(Wqkv, 3, axis=0)
    bq, bk, bv = np.split(bqkv, 3)
    q = (q_in @ Wq.T + bq).reshape(-1, H, dh)
    k = (kv_in @ Wk.T + bk).reshape(-1, H, dh)
    v = (kv_in @ Wv.T + bv).reshape(-1, H, dh)
    att = np.einsum("qhd,khd->hqk", q, k) / np.float32(np.sqrt(dh))
    att = att - att.max(-1, keepdims=True)
    att = np.exp(att)
    att /= att.sum(-1, keepdims=True)
    o = np.einsum("hqk,khd->qhd", att.astype(np.float32), v).reshape(-1, E)
    return o @ Wo.T + bo


def _reference_np(vision_feature, text_embed, attention_mask,
                  Wqkv1, bqkv1, Wo1, bo1, Wqkv2, bqkv2, Wo2, bo2,
                  Wqkvc, bqkvc, Woc, boc, Wf1, bf1, Wf2, bf2, Ws, bs):
    t, sel_idx, rem_idx = _score_partition(vision_feature, text_embed,
                                           attention_mask)
    sel = vision_feature[sel_idx]
    rem = vision_feature[rem_idx]
    cat = np.concatenate([sel, text_embed], axis=0)
    x = _ln_np(_mha_np(cat, cat, Wqkv1, bqkv1, Wo1, bo1) + cat)
    r = _ln_np(_mha_np(rem, rem, Wqkv2, bqkv2, Wo2, bo2) + rem)
    x = _ln_np(_mha_np(r, x, Wqkvc, bqkvc, Woc, boc) + r)
    ffn = _gelu_np(x @ Wf1.T + bf1) @ Wf2.T + bf2
    x = _ln_np(x + ffn)
    logits = (x @ Ws.T + bs).squeeze(-1)
    es = 1.0 / (1.0 + np.exp(-logits))
    k = int(t * EXPAND)
    ei = np.argsort(-es, kind="stable")[:k]
    final = np.sort(np.concatenate([sel_idx, rem_idx[ei]]))
    return vision_feature[final]


# ----------------------------------------------------------------------------
# device program
# ----------------------------------------------------------------------------

def _pad128(n):
    return ((n + 127) // 128) * 128


def _build_device(ncat_real, nrem_real, debug=False):
    import concourse.bacc as bacc
    import concourse.mybir as mybir
    import concourse.tile as tile

    dt = mybir.dt
    F32 = dt.float32
    F32R = dt.float32r
    BF16 = dt.bfloat16
    AF = mybir.ActivationFunctionType
    ALU = mybir.AluOpType

    ncat = _pad128(ncat_real)
    nrem = _pad128(nrem_real)
    JC = ncat // 128  # kv tiles for cat (2)
    JR = nrem // 128  # kv tiles for rem (4)

    nc = bacc.Bacc("TRN2", target_bir_lowering=False, debug=False,
                   num_devices=NCORES)

    # ---------------- DRAM I/O ----------------
    catT_d = nc.dram_tensor("catT", [E, ncat], F32R, kind="ExternalInput")
    remT_d = nc.dram_tensor("remT", [E, nrem], F32R, kind="ExternalInput")
    wd = {}
    for l in ("1", "2", "c"):
        for p in ("q", "k", "v"):
            wd[p + l] = nc.dram_tensor(f"w{p}{l}", [E, DLOC], F32R,
                                       kind="ExternalInput")
        wd["o" + l] = nc.dram_tensor(f"wo{l}", [DLOC, E], F32R,
                                     kind="ExternalInput")
    wd["f1"] = nc.dram_tensor("wf1", [E, FLOC], F32R, kind="ExternalInput")
    wd["f2"] = nc.dram_tensor("wf2", [FLOC, E], F32R, kind="ExternalInput")
    wd["s"] = nc.dram_tensor("ws", [128, ET], F32R, kind="ExternalInput")
    masks_d = nc.dram_tensor("masks", [128, 2], F32R, kind="ExternalInput")
    logits_d = nc.dram_tensor("logits", [1, nrem], F32, kind="ExternalOutput")
    dbg = {}
    if debug:
        for nm, L in (("dbg_x1", ncat), ("dbg_r", nrem), ("dbg_x2", nrem),
                      ("dbg_x3", nrem)):
            dbg[nm] = nc.dram_tensor(nm, [E, L], F32, kind="ExternalOutput")

    replica = [list(range(NCORES))]

    with tile.TileContext(nc, num_cores=NCORES) as tc:
        with (
            tc.tile_pool(name="acts", bufs=1) as acts,
            tc.tile_pool(name="psum", bufs=1, space="PSUM") as psum,
            tc.tile_pool(name="dram", bufs=1, space="DRAM") as dram,
        ):
            # ---- constants / packed stat tiles ----
            ones_col = acts.tile([128, 1], F32R, name="ones_col",
                                 tag="ones_col")
            nc.vector.memset(ones_col[:].bitcast(F32), 1.0)
            ones_row = acts.tile([1, 128], F32R, name="ones_row",
                                 tag="ones_row")
            nc.vector.memset(ones_row[:].bitcast(F32), 1.0)
            masks = acts.tile([128, 2], F32R, name="masks", tag="masks")
            nc.sync.dma_start(masks[:], masks_d.ap())

            def pp(name, L):
                return psum.tile([128, L], F32, name=name, tag="pp", bufs=8)

            def pstat(name, L):
                return psum.tile([1, L], F32, name=name, tag="pp", bufs=8)

            def wtile(name, cols):
                return acts.tile([128, cols], F32R, name=name, tag="wt",
                                 bufs=6, padded_shape=[128, 1024])

            # ---------------- building blocks ----------------
            def load_xT(name, dram_t, L, tagbase):
                ts = []
                for k in range(ET):
                    xt = acts.tile([128, L], F32R, name=f"{name}_{k}",
                                   tag=f"{tagbase}_{k}")
                    nc.sync.dma_start(xt[:],
                                      dram_t.ap()[128 * k:128 * (k + 1), :])
                    ts.append(xt)
                return ts

            def proj_fm(tagbase, w_dram, x_tiles, L, outtag):
                """q/k fm projection -> 4 tiles [128, L] (f32r)."""
                ps = [pp(f"ps_{tagbase}_{m}", L) for m in range(4)]
                outs = []
                for k in range(ET):
                    wt = wtile(f"w_{tagbase}_{k}", DLOC)
                    nc.sync.dma_start(
                        wt[:], w_dram.ap()[128 * k:128 * (k + 1), :])
                    for m in range(4):
                        nc.tensor.matmul(ps[m][:],
                                         wt[:, 128 * m:128 * (m + 1)],
                                         x_tiles[k][:],
                                         start=(k == 0), stop=(k == ET - 1))
                for m in range(4):
                    o = acts.tile([128, L], F32R, name=f"{tagbase}_{m}",
                                  tag=f"{outtag}_{m}")
                    nc.vector.tensor_copy(o[:], ps[m][:])
                    outs.append(o)
                return outs

            def proj_tm(tagbase, w_dram, x_tiles, L):
                """v tm projection -> L//128 tiles [128, DLOC] (f32r)."""
                jt = L // 128
                ps = [pp(f"ps_{tagbase}_{j}", DLOC) for j in range(jt)]
                outs = []
                for k in range(ET):
                    wt = wtile(f"w_{tagbase}_{k}", DLOC)
                    nc.sync.dma_start(
                        wt[:], w_dram.ap()[128 * k:128 * (k + 1), :])
                    for j in range(jt):
                        nc.tensor.matmul(ps[j][:],
                                         x_tiles[k][:, 128 * j:128 * (j + 1)],
                                         wt[:],
                                         start=(k == 0), stop=(k == ET - 1))
                for j in range(jt):
                    o = acts.tile([128, DLOC], F32R, name=f"{tagbase}_{j}",
                                  tag=f"v_{j}")
                    nc.vector.tensor_copy(o[:], ps[j][:])
                    outs.append(o)
                return outs

            def attention(tag, qT, kT, vT, Lq, Lkv, kv_valid, mask_idx):
                jt = Lkv // 128
                oT = []
                for h in range(HL):
                    exps = []
                    for j in range(jt):
                        p = pp(f"ps_s_{tag}_{h}_{j}", Lq)
                        for c in range(2):
                            nc.tensor.matmul(
                                p[:],
                                kT[2 * h + c][:, 128 * j:128 * (j + 1)],
                                qT[2 * h + c][:],
                                start=(c == 0), stop=(c == 1))
                        e = acts.tile([128, Lq], F32R,
                                      name=f"es_{tag}_{h}_{j}",
                                      tag=f"expS_{j}")
                        nc.scalar.activation(e[:], p[:], AF.Exp,
                                             scale=float(1.0 / np.sqrt(DH)))
                        exps.append(e)
                    dsum = pstat(f"ps_d_{tag}_{h}", Lq)
                    for j in range(jt):
                        if j == jt - 1 and kv_valid < Lkv:
                            col = masks[:, mask_idx:mask_idx + 1]
                        else:
                            col = ones_col[:]
                        nc.tensor.matmul(dsum[:], col, exps[j][:],
                                         start=(j == 0), stop=(j == jt - 1))
                    den = acts.tile([1, Lq], F32, name=f"den_{tag}_{h}",
                                    tag="aden")
                    rec = acts.tile([1, Lq], F32, name=f"rec_{tag}_{h}",
                                    tag="arec")
                    nc.vector.tensor_copy(den[:], dsum[:])
                    nc.vector.reciprocal(rec[:], den[:])
                    nc.vector.tensor_tensor(den[:], den[:], rec[:], ALU.mult)
                    nc.vector.tensor_scalar(den[:], den[:], -1.0, 2.0,
                                            ALU.mult, ALU.add)
                    rec2 = acts.tile([1, Lq], F32R, name=f"rec2_{tag}_{h}",
                                     tag="rec2")
                    nc.vector.tensor_tensor(rec2[:], rec[:], den[:], ALU.mult)
                    rrep_p = pp(f"ps_rr_{tag}_{h}", Lq)
                    nc.tensor.matmul(rrep_p[:], ones_row[:], rec2[:],
                                     start=True, stop=True)
                    rrep = acts.tile([128, Lq], F32, name=f"rr_{tag}_{h}",
                                     tag="rrep")
                    nc.scalar.copy(rrep[:], rrep_p[:])
                    for c in range(2):
                        po = pp(f"ps_o_{tag}_{h}_{c}", Lq)
                        for j in range(jt):
                            nc.tensor.matmul(
                                po[:],
                                vT[j][:, 256 * h + 128 * c:
                                      256 * h + 128 * (c + 1)],
                                exps[j][:],
                                start=(j == 0), stop=(j == jt - 1))
                        o = acts.tile([128, Lq], F32R,
                                      name=f"oT_{tag}_{h}_{c}",
                                      tag=f"oT_{2 * h + c}")
                        nc.vector.tensor_tensor(o[:], po[:], rrep[:],
                                                ALU.mult)
                        oT.append(o)
                return oT

            def out_proj_to_dram(tag, oT, w_dram, ar_in, Lq, sdt):
                for quarter in range(4):
                    wo_t = []
                    for k in range(4):
                        wt = wtile(f"wo_{tag}_{quarter}_{k}", 1024)
                        nc.sync.dma_start(
                            wt[:],
                            w_dram.ap()[128 * k:128 * (k + 1),
                                        1024 * quarter:1024 * (quarter + 1)])
                        wo_t.append(wt)
                    ps = []
                    for mm in range(8):
                        m = 8 * quarter + mm
                        ps.append(pp(f"ps_op_{tag}_{m}", Lq))
                    for k in range(4):
                        for mm in range(8):
                            nc.tensor.matmul(
                                ps[mm][:],
                                wo_t[k][:, 128 * mm:128 * (mm + 1)],
                                oT[k][:],
                                start=(k == 0), stop=(k == 3))
                    for mm in range(8):
                        m = 8 * quarter + mm
                        st = acts.tile([128, Lq], sdt,
                                       name=f"st_{tag}_{m}", tag="stage",
                                       bufs=3)
                        nc.vector.tensor_copy(st[:], ps[mm][:])
                        nc.sync.dma_start(
                            ar_in[128 * m:128 * (m + 1), :], st[:])

            def do_allreduce(tag, ar_in, ar_out):
                nc.gpsimd.collective_compute(
                    "AllReduce", ALU.add, replica_groups=replica,
                    ins=[ar_in.opt()], outs=[ar_out.opt()])

            def residual_ln(tag, ar_out, res_tiles, L, adt=F32, valid=None,
                            dump=None):
                """In-place: res_tiles[k] <- LN(ar_out + res_tiles)[k]."""
                # xsum (in-place into res slot)
                for k in range(ET):
                    b = acts.tile([128, L], adt, name=f"arb_{tag}_{k}",
                                  tag="arb", bufs=4)
                    nc.sync.dma_start(b[:], ar_out[128 * k:128 * (k + 1), :])
                    nc.vector.tensor_tensor(res_tiles[k][:], b[:],
                                            res_tiles[k][:], ALU.add)
                s1p = pstat(f"ps_s1_{tag}", L)
                s2p = pstat(f"ps_s2_{tag}", L)
                for k in range(ET):
                    nc.tensor.matmul(s1p[:], ones_col[:], res_tiles[k][:],
                                     start=(k == 0), stop=(k == ET - 1))
                for k in range(ET):
                    sq = acts.tile([128, L], F32R, name=f"sq_{tag}_{k}",
                                   tag="stage", bufs=3)
                    nc.scalar.square(sq[:], res_tiles[k][:])
                    nc.tensor.matmul(s2p[:], ones_col[:], sq[:],
                                     start=(k == 0), stop=(k == ET - 1))
                mean = acts.tile([1, L], F32, name=f"mean_{tag}",
                                 tag="lmean")
                var = acts.tile([1, L], F32, name=f"var_{tag}", tag="lvar")
                tmpa = acts.tile([1, L], F32, name=f"tmpa_{tag}", tag="ltmp")
                r0 = acts.tile([1, L], F32, name=f"r0_{tag}", tag="lr0")
                nc.scalar.mul(mean[:], s1p[:], 1.0 / E)
                nc.scalar.mul(var[:], s2p[:], 1.0 / E)
                nc.scalar.square(tmpa[:], mean[:])
                nc.vector.tensor_sub(var[:], var[:], tmpa[:])
                nc.vector.tensor_scalar_add(var[:], var[:], 1e-5)
                nc.scalar.sqrt(tmpa[:], var[:])
                nc.vector.reciprocal(r0[:], tmpa[:])
                nc.vector.tensor_tensor(tmpa[:], r0[:], r0[:], ALU.mult)
                nc.vector.tensor_tensor(tmpa[:], tmpa[:], var[:], ALU.mult)
                nc.vector.tensor_scalar(tmpa[:], tmpa[:], -0.5, 1.5, ALU.mult,
                                        ALU.add)
                rstd = acts.tile([1, L], F32R, name=f"rstd_{tag}", tag="rstd")
                nmr = acts.tile([1, L], F32R, name=f"nmr_{tag}", tag="nmr")
                nc.vector.tensor_tensor(rstd[:], r0[:], tmpa[:], ALU.mult)
                nc.vector.scalar_tensor_tensor(nmr[:], mean[:], -1.0, rstd[:],
                                               ALU.mult, ALU.mult)
                Apsum = pp(f"ps_A_{tag}", L)
                nc.tensor.matmul(Apsum[:], ones_row[:], rstd[:], start=True,
                                 stop=True)
                Bpsum = pp(f"ps_B_{tag}", L)
                nc.tensor.matmul(Bpsum[:], ones_row[:], nmr[:], start=True,
                                 stop=True)
                Asb = acts.tile([128, L], F32, name=f"A_{tag}", tag="Asb")
                nc.scalar.copy(Asb[:], Apsum[:])
                Bsb = acts.tile([128, L], F32, name=f"B_{tag}", tag="Bsb")
                nc.scalar.copy(Bsb[:], Bpsum[:])
                for k in range(ET):
                    nc.vector.tensor_tensor(res_tiles[k][:], res_tiles[k][:],
                                            Asb[:], ALU.mult)
                    nc.vector.tensor_tensor(res_tiles[k][:], res_tiles[k][:],
                                            Bsb[:], ALU.add)
                    if valid is not None and valid < L:
                        nc.vector.memset(
                            res_tiles[k][:, valid:L].bitcast(F32), 0.0)
                    if dump is not None:
                        nc.sync.dma_start(
                            dump.ap()[128 * k:128 * (k + 1), :],
                            res_tiles[k][:].bitcast(F32))
                return res_tiles

            # ================= program =================
            # fm512 family ("a_{k}"): remT -> r -> x2 -> x3 (in-place chain)
            # fm256 family ("b_{k}"): catT -> x1
            a_t = load_xT("remT", remT_d, nrem, "a")

            # ---- MHA2 (rem self-attention) ----
            q2 = proj_fm("q2", wd["q2"], a_t, nrem, "q")
            k2 = proj_fm("k2", wd["k2"], a_t, nrem, "k")
            v2 = proj_tm("v2", wd["v2"], a_t, nrem)
            o2 = attention("a2", q2, k2, v2, nrem, nrem, nrem_real, 1)
            arin2 = dram.tile([E, nrem], BF16, name="arin2", tag="arin2")
            arout2 = dram.tile([E, nrem], BF16, name="arout2", tag="arout2", addr_space="Shared")
            out_proj_to_dram("op2", o2, wd["o2"], arin2, nrem, BF16)
            do_allreduce("2", arin2, arout2)

            # ---- MHA1 (cat self-attention), overlaps AR2 ----
            b_t = load_xT("catT", catT_d, ncat, "b")
            q1 = proj_fm("q1", wd["q1"], b_t, ncat, "q")
            k1 = proj_fm("k1", wd["k1"], b_t, ncat, "k")
            v1 = proj_tm("v1", wd["v1"], b_t, ncat)
            o1 = attention("a1", q1, k1, v1, ncat, ncat, ncat_real, 0)
            arin1 = dram.tile([E, ncat], BF16, name="arin1", tag="arin1")
            arout1 = dram.tile([E, ncat], BF16, name="arout1", tag="arout1", addr_space="Shared")
            out_proj_to_dram("op1", o1, wd["o1"], arin1, ncat, BF16)
            do_allreduce("1", arin1, arout1)

            # ---- LN stages: r = LN(AR2 + rem); x1 = LN(AR1 + cat) ----
            r_t = residual_ln("r", arout2, a_t, nrem, adt=BF16,
                              dump=dbg.get("dbg_r"))
            x1_t = residual_ln("x1", arout1, b_t, ncat, adt=BF16,
                               valid=ncat_real, dump=dbg.get("dbg_x1"))

            # ---- MHAc (q from r, kv from x1) ----
            qc = proj_fm("qc", wd["qc"], r_t, nrem, "q")
            kc = proj_fm("kc", wd["kc"], x1_t, ncat, "k")
            vc = proj_tm("vc", wd["vc"], x1_t, ncat)
            oc = attention("ac", qc, kc, vc, nrem, ncat, ncat_real, 0)
            arinc = dram.tile([E, nrem], BF16, name="arinc", tag="arinc")
            aroutc = dram.tile([E, nrem], BF16, name="aroutc", tag="aroutc", addr_space="Shared")
            out_proj_to_dram("opc", oc, wd["oc"], arinc, nrem, BF16)
            do_allreduce("c", arinc, aroutc)
            x2_t = residual_ln("x2", aroutc, r_t, nrem, adt=BF16,
                               dump=dbg.get("dbg_x2"))

            # ---- FFN ----
            # f1: hT = gelu(Wf1_shard @ x2): 8 psums, single weight sweep
            ps_f1 = [pp(f"ps_f1_{m}", nrem) for m in range(8)]
            for k in range(ET):
                wt = wtile(f"w_f1_{k}", FLOC)
                nc.sync.dma_start(
                    wt[:], wd["f1"].ap()[128 * k:128 * (k + 1), :])
                for m in range(8):
                    nc.tensor.matmul(ps_f1[m][:],
                                     wt[:, 128 * m:128 * (m + 1)],
                                     x2_t[k][:],
                                     start=(k == 0), stop=(k == ET - 1))
            hT = []
            for m in range(8):
                tg = f"v_{m}" if m < 4 else f"q_{m - 4}"
                h = acts.tile([128, nrem], F32R, name=f"hT_{m}", tag=tg)
                nc.scalar.activation(h[:], ps_f1[m][:], AF.Gelu)
                hT.append(h)
            # f2: quarters of output cols; psum group of 8 m-tiles per quarter
            arin4 = dram.tile([E, nrem], F32, name="arin4", tag="arin4")
            arout4 = dram.tile([E, nrem], F32, name="arout4", tag="arout4", addr_space="Shared")
            HK = FLOC // 128  # 8
            for quarter in range(4):
                ps = []
                for mm in range(8):
                    m = 8 * quarter + mm
                    ps.append(pp(f"ps_f2_{m}", nrem))
                for khalf in range(2):
                    wf_t = []
                    for kk in range(4):
                        k = 4 * khalf + kk
                        wt = wtile(f"w_f2_{quarter}_{k}", 1024)
                        nc.sync.dma_start(
                            wt[:],
                            wd["f2"].ap()[128 * k:128 * (k + 1),
                                          1024 * quarter:1024 * (quarter + 1)])
                        wf_t.append(wt)
                    for kk in range(4):
                        k = 4 * khalf + kk
                        for mm in range(8):
                            nc.tensor.matmul(
                                ps[mm][:],
                                wf_t[kk][:, 128 * mm:128 * (mm + 1)],
                                hT[k][:],
                                start=(k == 0), stop=(k == HK - 1))
                for mm in range(8):
                    m = 8 * quarter + mm
                    st = acts.tile([128, nrem], F32, name=f"st_f2_{m}",
                                   tag="stage", bufs=3)
                    nc.vector.tensor_copy(st[:], ps[mm][:])
                    nc.sync.dma_start(arin4[128 * m:128 * (m + 1), :], st[:])
            do_allreduce("4", arin4, arout4)
            x3_t = residual_ln("x3", arout4, x2_t, nrem,
                               dump=dbg.get("dbg_x3"))

            # ---- logits ----
            ws_sb = acts.tile([128, ET], F32R, name="ws_sb", tag="ws_sb")
            nc.sync.dma_start(ws_sb[:], wd["s"].ap())
            lp = pstat("ps_logit", nrem)
            for k in range(ET):
                nc.tensor.matmul(lp[:], ws_sb[:, k:k + 1], x3_t[k][:],
                                 start=(k == 0), stop=(k == ET - 1))
            lsb = acts.tile([1, nrem], F32, name="lsb", tag="lsb")
            nc.vector.tensor_copy(lsb[:], lp[:])
            nc.sync.dma_start(logits_d.ap(), lsb[:])

    nc.compile()
    return nc


# ----------------------------------------------------------------------------
# host orchestration
# ----------------------------------------------------------------------------

def _prep_in_maps(vision_feature, text_embed, sel_idx, rem_idx, ncat, nrem,
                  Wqkv1, Wo1, Wqkv2, Wo2, Wqkvc, Woc, Wf1, Wf2, Ws):
    f32 = np.float32
    sel = vision_feature[sel_idx]
    rem = vision_feature[rem_idx]
    cat = np.concatenate([sel, text_embed], axis=0)
    catT = np.zeros((E, ncat), f32)
    catT[:, :cat.shape[0]] = cat.T
    remT = np.zeros((E, nrem), f32)
    remT[:, :rem.shape[0]] = rem.T

    ncat_real = cat.shape[0]
    nrem_real = rem.shape[0]
    masks = np.zeros((128, 2), f32)
    masks[:ncat_real - 128 * (ncat // 128 - 1), 0] = 1.0
    masks[:nrem_real - 128 * (nrem // 128 - 1), 1] = 1.0

    in_maps = []
    for c in range(NCORES):
        hs = slice(DLOC * c, DLOC * (c + 1))
        fs = slice(FLOC * c, FLOC * (c + 1))
        m = {"catT": catT, "remT": remT, "masks": masks,
             "ws": np.ascontiguousarray(Ws[0].reshape(ET, 128).T)}
        for l, Wqkv, Wo in (("1", Wqkv1, Wo1), ("2", Wqkv2, Wo2),
                            ("c", Wqkvc, Woc)):
            Wq, Wk, Wv = Wqkv[:E], Wqkv[E:2 * E], Wqkv[2 * E:]
            m["wq" + l] = np.ascontiguousarray(Wq[hs].T)
            m["wk" + l] = np.ascontiguousarray(Wk[hs].T)
            m["wv" + l] = np.ascontiguousarray(Wv[hs].T)
            m["wo" + l] = np.ascontiguousarray(Wo[:, hs].T)
        m["wf1"] = np.ascontiguousarray(Wf1[fs].T)
        m["wf2"] = np.ascontiguousarray(Wf2[:, fs].T)
        in_maps.append(m)
    return in_maps


def run_device(in_maps, ncat_real, nrem_real, debug=False, trace=False):
    from concourse.bass_utils import run_bass_kernel_spmd

    key = (ncat_real, nrem_real, debug)
    if key not in _CACHE:
        _CACHE[key] = _build_device(ncat_real, nrem_real, debug=debug)
    nc = _CACHE[key]
    return run_bass_kernel_spmd(nc, in_maps, list(range(NCORES)), trace=trace)


def _kernel_impl(inputs, debug=False, trace=False):
    vision_feature = np.asarray(inputs["vision_feature"], np.float32)
    text_embed = np.asarray(inputs["text_embed"], np.float32)
    attention_mask = np.asarray(inputs["attention_mask"])

    biases_zero = all(
        not np.any(np.asarray(inputs[b]))
        for b in ("bqkv1", "bo1", "bqkv2", "bo2", "bqkvc", "boc",
                  "bf1", "bf2", "bs"))
    if (not bool(attention_mask.all())) or (not biases_zero):
        return _reference_np(**{k: np.asarray(v) for k, v in inputs.items()}), None

    t, sel_idx, rem_idx = _score_partition(vision_feature, text_embed,
                                           attention_mask)
    ncat_real = t + text_embed.shape[0]
    nrem_real = vision_feature.shape[0] - t
    kk = int(t * EXPAND)

    in_maps = _prep_in_maps(
        vision_feature, text_embed, sel_idx, rem_idx,
        _pad128(ncat_real), _pad128(nrem_real),
        np.asarray(inputs["Wqkv1"], np.float32),
        np.asarray(inputs["Wo1"], np.float32),
        np.asarray(inputs["Wqkv2"], np.float32),
        np.asarray(inputs["Wo2"], np.float32),
        np.asarray(inputs["Wqkvc"], np.float32),
        np.asarray(inputs["Woc"], np.float32),
        np.asarray(inputs["Wf1"], np.float32),
        np.asarray(inputs["Wf2"], np.float32),
        np.asarray(inputs["Ws"], np.float32))
    res = run_device(in_maps, ncat_real, nrem_real, debug=debug, trace=trace)
    logits = res.results[0]["logits"][0, :nrem_real]
    es = (1.0 / (1.0 + np.exp(-logits.astype(np.float32))))
    ei = np.argsort(-es, kind="stable")[:kk]
    final = np.sort(np.concatenate([sel_idx, rem_idx[ei]]))
    return vision_feature[final], res


def kernel(**inputs):
    out, _ = _kernel_impl(inputs)
    return out


# revision 13
# speedup vs baseline: 1.1455x; 1.0704x over previous
"""Trainium2 Bass kernel for nn_CosSimRouter_learn_49778670960796.

Host: cosine-similarity scoring / sort / gather (tiny, shape-determining).
Device (8 NeuronCores, tensor-parallel over heads/hidden):
  3x MHA + FFN + logits; fp32 storage, float32r matmuls; AllReduce after
  out-proj / FFN2 (Megatron-style TP). Activations feature-major [E, L].
Host: top-k + final gather (exact rows of the input).
"""

import numpy as np

E = 4096
H = 16
HID = 8192
GAMMA = 0.2
TEMP = 0.05
EXPAND = 0.7
NCORES = 8
ET = E // 128  # 32 feature tiles
DH = E // H  # 256
HL = H // NCORES  # 2 heads per core
DLOC = HL * DH  # 512 local head dims
FLOC = HID // NCORES  # 1024 local ffn hidden

_CACHE = {}


# ----------------------------------------------------------------------------
# host-side reference math (numpy, fp32) for the scoring stage + fallback
# ----------------------------------------------------------------------------

def _score_partition(vision_feature, text_embed, attention_mask):
    vf = vision_feature.astype(np.float32)
    te = text_embed.astype(np.float32)
    vn = vf / np.maximum(np.linalg.norm(vf, axis=-1, keepdims=True), 1e-8)
    tn = te / np.maximum(np.linalg.norm(te, axis=-1, keepdims=True), 1e-8)
    cs = vn @ tn.T
    cs = np.where(attention_mask[None, :], cs, np.float32(0.0))
    m = cs.max(axis=-1) / np.float32(TEMP)
    e = np.exp(m - m.max())
    scores = e / e.sum()
    order = np.argsort(-scores, kind="stable")
    cum = np.cumsum(scores[order])
    t = int((cum <= GAMMA).sum())
    return t, order[:t], order[t:]


def _ln_np(x):
    m = x.mean(-1, keepdims=True)
    v = ((x - m) ** 2).mean(-1, keepdims=True)
    return (x - m) / np.sqrt(v + 1e-5)


def _gelu_np(x):
    import math

    erf = np.frompyfunc(math.erf, 1, 1)
    return (x * 0.5 * (1.0 + erf(x / math.sqrt(2.0)).astype(np.float64))
            ).astype(x.dtype)


def _mha_np(q_in, kv_in, Wqkv, bqkv, Wo, bo):
    dh = E // H
    Wq, Wk, Wv = np.split(Wqkv, 3, axis=0)
    bq, bk, bv = np.split(bqkv, 3)
    q = (q_in @ Wq.T + bq).reshape(-1, H, dh)
    k = (kv_in @ Wk.T + bk).reshape(-1, H, dh)
    v = (kv_in @ Wv.T + bv).reshape(-1, H, dh)
    att = np.einsum("qhd,khd->hqk", q, k) / np.float32(np.sqrt(dh))
    att = att - att.max(-1, keepdims=True)
    att = np.exp(att)
    att /= att.sum(-1, keepdims=True)
    o = np.einsum("hqk,khd->qhd", att.astype(np.float32), v).reshape(-1, E)
    return o @ Wo.T + bo


def _reference_np(vision_feature, text_embed, attention_mask,
                  Wqkv1, bqkv1, Wo1, bo1, Wqkv2, bqkv2, Wo2, bo2,
                  Wqkvc, bqkvc, Woc, boc, Wf1, bf1, Wf2, bf2, Ws, bs):
    t, sel_idx, rem_idx = _score_partition(vision_feature, text_embed,
                                           attention_mask)
    sel = vision_feature[sel_idx]
    rem = vision_feature[rem_idx]
    cat = np.concatenate([sel, text_embed], axis=0)
    x = _ln_np(_mha_np(cat, cat, Wqkv1, bqkv1, Wo1, bo1) + cat)
    r = _ln_np(_mha_np(rem, rem, Wqkv2, bqkv2, Wo2, bo2) + rem)
    x = _ln_np(_mha_np(r, x, Wqkvc, bqkvc, Woc, boc) + r)
    ffn = _gelu_np(x @ Wf1.T + bf1) @ Wf2.T + bf2
    x = _ln_np(x + ffn)
    logits = (x @ Ws.T + bs).squeeze(-1)
    es = 1.0 / (1.0 + np.exp(-logits))
    k = int(t * EXPAND)
    ei = np.argsort(-es, kind="stable")[:k]
    final = np.sort(np.concatenate([sel_idx, rem_idx[ei]]))
    return vision_feature[final]


# ----------------------------------------------------------------------------
# device program
# ----------------------------------------------------------------------------

def _pad128(n):
    return ((n + 127) // 128) * 128


def _build_device(ncat_real, nrem_real, debug=False):
    import concourse.bacc as bacc
    import concourse.mybir as mybir
    import concourse.tile as tile

    dt = mybir.dt
    F32 = dt.float32
    F32R = dt.float32r
    BF16 = dt.bfloat16
    AF = mybir.ActivationFunctionType
    ALU = mybir.AluOpType

    ncat = _pad128(ncat_real)
    nrem = _pad128(nrem_real)
    JC = ncat // 128  # kv tiles for cat (2)
    JR = nrem // 128  # kv tiles for rem (4)

    nc = bacc.Bacc("TRN2", target_bir_lowering=False, debug=False,
                   num_devices=NCORES)

    # ---------------- DRAM I/O ----------------
    catT_d = nc.dram_tensor("catT", [E, ncat], F32R, kind="ExternalInput")
    remT_d = nc.dram_tensor("remT", [E, nrem], F32R, kind="ExternalInput")
    wd = {}
    for l in ("1", "2", "c"):
        for p in ("q", "k", "v"):
            wd[p + l] = nc.dram_tensor(f"w{p}{l}", [E, DLOC], F32R,
                                       kind="ExternalInput")
        wd["o" + l] = nc.dram_tensor(f"wo{l}", [DLOC, E], F32R,
                                     kind="ExternalInput")
    wd["f1"] = nc.dram_tensor("wf1", [E, FLOC], F32R, kind="ExternalInput")
    wd["f2"] = nc.dram_tensor("wf2", [FLOC, E], F32R, kind="ExternalInput")
    wd["s"] = nc.dram_tensor("ws", [128, ET], F32R, kind="ExternalInput")
    masks_d = nc.dram_tensor("masks", [128, 4], F32R, kind="ExternalInput")
    logits_d = nc.dram_tensor("logits", [1, nrem], F32, kind="ExternalOutput")
    dbg = {}
    if debug:
        for nm, L in (("dbg_x1", ncat), ("dbg_r", nrem), ("dbg_x2", nrem),
                      ("dbg_x3", nrem)):
            dbg[nm] = nc.dram_tensor(nm, [E, L], F32, kind="ExternalOutput")

    replica = [list(range(NCORES))]

    with tile.TileContext(nc, num_cores=NCORES) as tc:
        with (
            tc.tile_pool(name="acts", bufs=1) as acts,
            tc.tile_pool(name="psum", bufs=1, space="PSUM") as psum,
            tc.tile_pool(name="dram", bufs=1, space="DRAM") as dram,
        ):
            # ---- constants / packed stat tiles ----
            ones_col = acts.tile([128, 1], F32R, name="ones_col",
                                 tag="ones_col")
            nc.vector.memset(ones_col[:].bitcast(F32), 1.0)
            ones_row = acts.tile([1, 128], F32R, name="ones_row",
                                 tag="ones_row")
            nc.vector.memset(ones_row[:].bitcast(F32), 1.0)
            masks = acts.tile([128, 4], F32R, name="masks", tag="masks")
            nc.sync.dma_start(masks[:], masks_d.ap())

            def pp(name, L):
                return psum.tile([128, L], F32, name=name, tag="pp", bufs=8)

            def pstat(name, L):
                return psum.tile([1, L], F32, name=name, tag="pp", bufs=8)

            def wtile(name, cols):
                return acts.tile([128, cols], F32R, name=name, tag="wt",
                                 bufs=6, padded_shape=[128, 1024])

            # ---------------- building blocks ----------------
            def load_xT(name, dram_t, L, tagbase):
                ts = []
                for k in range(ET):
                    xt = acts.tile([128, L], F32R, name=f"{name}_{k}",
                                   tag=f"{tagbase}_{k}")
                    nc.sync.dma_start(xt[:],
                                      dram_t.ap()[128 * k:128 * (k + 1), :])
                    ts.append(xt)
                return ts

            def proj_fm(tagbase, w_dram, x_tiles, L, outtag):
                """q/k fm projection -> 4 tiles [128, L] (f32r)."""
                ps = [pp(f"ps_{tagbase}_{m}", L) for m in range(4)]
                outs = []
                for k in range(ET):
                    wt = wtile(f"w_{tagbase}_{k}", DLOC)
                    nc.sync.dma_start(
                        wt[:], w_dram.ap()[128 * k:128 * (k + 1), :])
                    for m in range(4):
                        nc.tensor.matmul(ps[m][:],
                                         wt[:, 128 * m:128 * (m + 1)],
                                         x_tiles[k][:],
                                         start=(k == 0), stop=(k == ET - 1))
                for m in range(4):
                    o = acts.tile([128, L], F32R, name=f"{tagbase}_{m}",
                                  tag=f"{outtag}_{m}")
                    nc.vector.tensor_copy(o[:], ps[m][:])
                    outs.append(o)
                return outs

            def proj_tm(tagbase, w_dram, x_tiles, L):
                """v tm projection -> L//128 tiles [128, DLOC] (f32r)."""
                jt = L // 128
                ps = [pp(f"ps_{tagbase}_{j}", DLOC) for j in range(jt)]
                outs = []
                for k in range(ET):
                    wt = wtile(f"w_{tagbase}_{k}", DLOC)
                    nc.sync.dma_start(
                        wt[:], w_dram.ap()[128 * k:128 * (k + 1), :])
                    for j in range(jt):
                        nc.tensor.matmul(ps[j][:],
                                         x_tiles[k][:, 128 * j:128 * (j + 1)],
                                         wt[:],
                                         start=(k == 0), stop=(k == ET - 1))
                for j in range(jt):
                    o = acts.tile([128, DLOC], F32R, name=f"{tagbase}_{j}",
                                  tag=f"v_{j}")
                    nc.vector.tensor_copy(o[:], ps[j][:])
                    outs.append(o)
                return outs

            def attention(tag, qT, kT, vT, Lq, Lkv, kv_valid, mask_idx):
                jt = Lkv // 128
                oT = []
                for h in range(HL):
                    exps = []
                    for j in range(jt):
                        p = pp(f"ps_s_{tag}_{h}_{j}", Lq)
                        for c in range(2):
                            nc.tensor.matmul(
                                p[:],
                                kT[2 * h + c][:, 128 * j:128 * (j + 1)],
                                qT[2 * h + c][:],
                                start=(c == 0), stop=(c == 1))
                        e = acts.tile([128, Lq], F32R,
                                      name=f"es_{tag}_{h}_{j}",
                                      tag=f"expS_{j}")
                        nc.scalar.activation(e[:], p[:], AF.Exp,
                                             scale=float(1.0 / np.sqrt(DH)))
                        exps.append(e)
                    dsum = pstat(f"ps_d_{tag}_{h}", Lq)
                    for j in range(jt):
                        if j == jt - 1 and kv_valid < Lkv:
                            col = masks[:, mask_idx:mask_idx + 1]
                        else:
                            col = ones_col[:]
                        nc.tensor.matmul(dsum[:], col, exps[j][:],
                                         start=(j == 0), stop=(j == jt - 1))
                    den = acts.tile([1, Lq], F32, name=f"den_{tag}_{h}",
                                    tag="aden")
                    rec = acts.tile([1, Lq], F32, name=f"rec_{tag}_{h}",
                                    tag="arec")
                    nc.vector.tensor_copy(den[:], dsum[:])
                    nc.vector.reciprocal(rec[:], den[:])
                    nc.vector.tensor_tensor(den[:], den[:], rec[:], ALU.mult)
                    nc.vector.tensor_scalar(den[:], den[:], -1.0, 2.0,
                                            ALU.mult, ALU.add)
                    rec2 = acts.tile([1, Lq], F32R, name=f"rec2_{tag}_{h}",
                                     tag="rec2")
                    nc.vector.tensor_tensor(rec2[:], rec[:], den[:], ALU.mult)
                    rrep_p = pp(f"ps_rr_{tag}_{h}", Lq)
                    nc.tensor.matmul(rrep_p[:], ones_row[:], rec2[:],
                                     start=True, stop=True)
                    rrep = acts.tile([128, Lq], F32, name=f"rr_{tag}_{h}",
                                     tag="rrep")
                    nc.scalar.copy(rrep[:], rrep_p[:])
                    for c in range(2):
                        po = pp(f"ps_o_{tag}_{h}_{c}", Lq)
                        for j in range(jt):
                            nc.tensor.matmul(
                                po[:],
                                vT[j][:, 256 * h + 128 * c:
                                      256 * h + 128 * (c + 1)],
                                exps[j][:],
                                start=(j == 0), stop=(j == jt - 1))
                        o = acts.tile([128, Lq], F32R,
                                      name=f"oT_{tag}_{h}_{c}",
                                      tag=f"oT_{2 * h + c}")
                        nc.vector.tensor_tensor(o[:], po[:], rrep[:],
                                                ALU.mult)
                        oT.append(o)
                return oT

            def out_proj_to_dram(tag, oT, w_dram, ar_in, Lq, sdt):
                for quarter in range(4):
                    wo_t = []
                    for k in range(4):
                        wt = wtile(f"wo_{tag}_{quarter}_{k}", 1024)
                        nc.sync.dma_start(
                            wt[:],
                            w_dram.ap()[128 * k:128 * (k + 1),
                                        1024 * quarter:1024 * (quarter + 1)])
                        wo_t.append(wt)
                    ps = []
                    for mm in range(8):
                        m = 8 * quarter + mm
                        ps.append(pp(f"ps_op_{tag}_{m}", Lq))
                    for k in range(4):
                        for mm in range(8):
                            nc.tensor.matmul(
                                ps[mm][:],
                                wo_t[k][:, 128 * mm:128 * (mm + 1)],
                                oT[k][:],
                                start=(k == 0), stop=(k == 3))
                    for mm in range(8):
                        m = 8 * quarter + mm
                        st = acts.tile([128, Lq], sdt,
                                       name=f"st_{tag}_{m}", tag="stage",
                                       bufs=3)
                        nc.vector.tensor_copy(st[:], ps[mm][:])
                        nc.sync.dma_start(
                            ar_in[128 * m:128 * (m + 1), :], st[:])

            def do_allreduce(tag, ar_in, ar_out):
                nc.gpsimd.collective_compute(
                    "AllReduce", ALU.add, replica_groups=replica,
                    ins=[ar_in.opt()], outs=[ar_out.opt()])

            def residual_ln(tag, ar_out, res_tiles, L, adt=F32, valid=None,
                            dump=None, normalize=True):
                """In-place: res_tiles[k] <- LN(ar_out + res_tiles)[k]."""
                # xsum (in-place into res slot)
                for k in range(ET):
                    b = acts.tile([128, L], adt, name=f"arb_{tag}_{k}",
                                  tag="arb", bufs=4)
                    nc.sync.dma_start(b[:], ar_out[128 * k:128 * (k + 1), :])
                    nc.vector.tensor_tensor(res_tiles[k][:], b[:],
                                            res_tiles[k][:], ALU.add)
                s1p = pstat(f"ps_s1_{tag}", L)
                s2p = pstat(f"ps_s2_{tag}", L)
                for k in range(ET):
                    nc.tensor.matmul(s1p[:], ones_col[:], res_tiles[k][:],
                                     start=(k == 0), stop=(k == ET - 1))
                for k in range(ET):
                    sq = acts.tile([128, L], F32R, name=f"sq_{tag}_{k}",
                                   tag="stage", bufs=3)
                    nc.scalar.square(sq[:], res_tiles[k][:])
                    nc.tensor.matmul(s2p[:], ones_col[:], sq[:],
                                     start=(k == 0), stop=(k == ET - 1))
                mean = acts.tile([1, L], F32, name=f"mean_{tag}",
                                 tag="lmean")
                var = acts.tile([1, L], F32, name=f"var_{tag}", tag="lvar")
                tmpa = acts.tile([1, L], F32, name=f"tmpa_{tag}", tag="ltmp")
                r0 = acts.tile([1, L], F32, name=f"r0_{tag}", tag="lr0")
                nc.scalar.mul(mean[:], s1p[:], 1.0 / E)
                nc.scalar.mul(var[:], s2p[:], 1.0 / E)
                nc.scalar.square(tmpa[:], mean[:])
                nc.vector.tensor_sub(var[:], var[:], tmpa[:])
                nc.vector.tensor_scalar_add(var[:], var[:], 1e-5)
                nc.scalar.sqrt(tmpa[:], var[:])
                nc.vector.reciprocal(r0[:], tmpa[:])
                nc.vector.tensor_tensor(tmpa[:], r0[:], r0[:], ALU.mult)
                nc.vector.tensor_tensor(tmpa[:], tmpa[:], var[:], ALU.mult)
                nc.vector.tensor_scalar(tmpa[:], tmpa[:], -0.5, 1.5, ALU.mult,
                                        ALU.add)
                rstd = acts.tile([1, L], F32R, name=f"rstd_{tag}", tag="rstd")
                nmr = acts.tile([1, L], F32R, name=f"nmr_{tag}", tag="nmr")
                nc.vector.tensor_tensor(rstd[:], r0[:], tmpa[:], ALU.mult)
                nc.vector.scalar_tensor_tensor(nmr[:], mean[:], -1.0, rstd[:],
                                               ALU.mult, ALU.mult)
                if not normalize:
                    return rstd, nmr
                Apsum = pp(f"ps_A_{tag}", L)
                nc.tensor.matmul(Apsum[:], ones_row[:], rstd[:], start=True,
                                 stop=True)
                Bpsum = pp(f"ps_B_{tag}", L)
                nc.tensor.matmul(Bpsum[:], ones_row[:], nmr[:], start=True,
                                 stop=True)
                Asb = acts.tile([128, L], F32, name=f"A_{tag}", tag="Asb")
                nc.scalar.copy(Asb[:], Apsum[:])
                Bsb = acts.tile([128, L], F32, name=f"B_{tag}", tag="Bsb")
                nc.scalar.copy(Bsb[:], Bpsum[:])
                for k in range(ET):
                    nc.vector.tensor_tensor(res_tiles[k][:], res_tiles[k][:],
                                            Asb[:], ALU.mult)
                    nc.vector.tensor_tensor(res_tiles[k][:], res_tiles[k][:],
                                            Bsb[:], ALU.add)
                    if valid is not None and valid < L:
                        nc.vector.memset(
                            res_tiles[k][:, valid:L].bitcast(F32), 0.0)
                    if dump is not None:
                        nc.sync.dma_start(
                            dump.ap()[128 * k:128 * (k + 1), :],
                            res_tiles[k][:].bitcast(F32))
                return res_tiles

            # ================= program =================
            # fm512 family ("a_{k}"): remT -> r -> x2 -> x3 (in-place chain)
            # fm256 family ("b_{k}"): catT -> x1
            a_t = load_xT("remT", remT_d, nrem, "a")

            # ---- MHA2 (rem self-attention) ----
            q2 = proj_fm("q2", wd["q2"], a_t, nrem, "q")
            k2 = proj_fm("k2", wd["k2"], a_t, nrem, "k")
            v2 = proj_tm("v2", wd["v2"], a_t, nrem)
            o2 = attention("a2", q2, k2, v2, nrem, nrem, nrem_real, 1)
            arin2 = dram.tile([E, nrem], BF16, name="arin2", tag="arin2")
            arout2 = dram.tile([E, nrem], BF16, name="arout2", tag="arout2", addr_space="Shared")
            out_proj_to_dram("op2", o2, wd["o2"], arin2, nrem, BF16)
            do_allreduce("2", arin2, arout2)

            # ---- MHA1 (cat self-attention), overlaps AR2 ----
            b_t = load_xT("catT", catT_d, ncat, "b")
            q1 = proj_fm("q1", wd["q1"], b_t, ncat, "q")
            k1 = proj_fm("k1", wd["k1"], b_t, ncat, "k")
            v1 = proj_tm("v1", wd["v1"], b_t, ncat)
            o1 = attention("a1", q1, k1, v1, ncat, ncat, ncat_real, 0)
            arin1 = dram.tile([E, ncat], BF16, name="arin1", tag="arin1")
            arout1 = dram.tile([E, ncat], BF16, name="arout1", tag="arout1", addr_space="Shared")
            out_proj_to_dram("op1", o1, wd["o1"], arin1, ncat, BF16)
            do_allreduce("1", arin1, arout1)

            # ---- LN stages: r = LN(AR2 + rem); x1 = LN(AR1 + cat) ----
            r_t = residual_ln("r", arout2, a_t, nrem, adt=BF16,
                              dump=dbg.get("dbg_r"))
            x1_t = residual_ln("x1", arout1, b_t, ncat, adt=BF16,
                               valid=ncat_real, dump=dbg.get("dbg_x1"))

            # ---- MHAc (q from r, kv from x1) ----
            qc = proj_fm("qc", wd["qc"], r_t, nrem, "q")
            kc = proj_fm("kc", wd["kc"], x1_t, ncat, "k")
            vc = proj_tm("vc", wd["vc"], x1_t, ncat)
            oc = attention("ac", qc, kc, vc, nrem, ncat, ncat_real, 0)
            arinc = dram.tile([E, nrem], BF16, name="arinc", tag="arinc")
            aroutc = dram.tile([E, nrem], BF16, name="aroutc", tag="aroutc", addr_space="Shared")
            out_proj_to_dram("opc", oc, wd["oc"], arinc, nrem, BF16)
            do_allreduce("c", arinc, aroutc)
            x2_t = residual_ln("x2", aroutc, r_t, nrem, adt=BF16,
                               dump=dbg.get("dbg_x2"))

            # ---- FFN ----
            # f1: hT = gelu(Wf1_shard @ x2): 8 psums, single weight sweep
            ps_f1 = [pp(f"ps_f1_{m}", nrem) for m in range(8)]
            for k in range(ET):
                wt = wtile(f"w_f1_{k}", FLOC)
                nc.sync.dma_start(
                    wt[:], wd["f1"].ap()[128 * k:128 * (k + 1), :])
                for m in range(8):
                    nc.tensor.matmul(ps_f1[m][:],
                                     wt[:, 128 * m:128 * (m + 1)],
                                     x2_t[k][:],
                                     start=(k == 0), stop=(k == ET - 1))
            hT = []
            for m in range(8):
                tg = f"v_{m}" if m < 4 else f"q_{m - 4}"
                h = acts.tile([128, nrem], F32R, name=f"hT_{m}", tag=tg)
                nc.scalar.activation(h[:], ps_f1[m][:], AF.Gelu)
                hT.append(h)
            # f2: quarters of output cols; psum group of 8 m-tiles per quarter
            arin4 = dram.tile([E, nrem], F32, name="arin4", tag="arin4")
            arout4 = dram.tile([E, nrem], F32, name="arout4", tag="arout4", addr_space="Shared")
            HK = FLOC // 128  # 8
            for quarter in range(4):
                ps = []
                for mm in range(8):
                    m = 8 * quarter + mm
                    ps.append(pp(f"ps_f2_{m}", nrem))
                for khalf in range(2):
                    wf_t = []
                    for kk in range(4):
                        k = 4 * khalf + kk
                        wt = wtile(f"w_f2_{quarter}_{k}", 1024)
                        nc.sync.dma_start(
                            wt[:],
                            wd["f2"].ap()[128 * k:128 * (k + 1),
                                          1024 * quarter:1024 * (quarter + 1)])
                        wf_t.append(wt)
                    for kk in range(4):
                        k = 4 * khalf + kk
                        for mm in range(8):
                            nc.tensor.matmul(
                                ps[mm][:],
                                wf_t[kk][:, 128 * mm:128 * (mm + 1)],
                                hT[k][:],
                                start=(k == 0), stop=(k == HK - 1))
                for mm in range(8):
                    m = 8 * quarter + mm
                    st = acts.tile([128, nrem], F32, name=f"st_f2_{m}",
                                   tag="stage", bufs=3)
                    nc.vector.tensor_copy(st[:], ps[mm][:])
                    nc.sync.dma_start(arin4[128 * m:128 * (m + 1), :], st[:])
            do_allreduce("4", arin4, arout4)
            rstd3, nmr3 = residual_ln("x3", arout4, x2_t, nrem,
                                      normalize=(dbg.get("dbg_x3")
                                                 is not None),
                                      dump=dbg.get("dbg_x3"))

            # ---- logits fused with the affine LN ----
            ws_sb = acts.tile([128, ET], F32R, name="ws_sb", tag="ws_sb")
            nc.sync.dma_start(ws_sb[:], wd["s"].ap())
            lp = pstat("ps_logit", nrem)
            for k in range(ET):
                nc.tensor.matmul(lp[:], ws_sb[:, k:k + 1], x2_t[k][:],
                                 start=(k == 0), stop=(k == ET - 1))
            wsd = acts.tile([1, nrem], F32, name="wsd", tag="wsd")
            nc.vector.tensor_copy(wsd[:], lp[:])
            lsb = acts.tile([1, nrem], F32, name="lsb", tag="lsb")
            if debug:
                # x2_t was normalized in-place; Ws @ xhat is the logit
                nc.vector.tensor_copy(lsb[:], wsd[:])
            else:
                wdot = acts.tile([1, nrem], F32, name="wdot", tag="wdot")
                nc.vector.tensor_tensor(wdot[:], rstd3[:], wsd[:], ALU.mult)
                nc.vector.scalar_tensor_tensor(lsb[:], nmr3[:],
                                               masks[0:1, 2:3], wdot[:],
                                               ALU.mult, ALU.add)
            nc.sync.dma_start(logits_d.ap(), lsb[:])

    nc.compile()
    return nc


# ----------------------------------------------------------------------------
# host orchestration
# ----------------------------------------------------------------------------

def _prep_in_maps(vision_feature, text_embed, sel_idx, rem_idx, ncat, nrem,
                  Wqkv1, Wo1, Wqkv2, Wo2, Wqkvc, Woc, Wf1, Wf2, Ws):
    f32 = np.float32
    sel = vision_feature[sel_idx]
    rem = vision_feature[rem_idx]
    cat = np.concatenate([sel, text_embed], axis=0)
    catT = np.zeros((E, ncat), f32)
    catT[:, :cat.shape[0]] = cat.T
    remT = np.zeros((E, nrem), f32)
    remT[:, :rem.shape[0]] = rem.T

    ncat_real = cat.shape[0]
    nrem_real = rem.shape[0]
    masks = np.zeros((128, 4), f32)
    masks[:ncat_real - 128 * (ncat // 128 - 1), 0] = 1.0
    masks[:nrem_real - 128 * (nrem // 128 - 1), 1] = 1.0
    masks[0, 2] = Ws.astype(np.float64).sum()

    in_maps = []
    for c in range(NCORES):
        hs = slice(DLOC * c, DLOC * (c + 1))
        fs = slice(FLOC * c, FLOC * (c + 1))
        m = {"catT": catT, "remT": remT, "masks": masks,
             "ws": np.ascontiguousarray(Ws[0].reshape(ET, 128).T)}
        for l, Wqkv, Wo in (("1", Wqkv1, Wo1), ("2", Wqkv2, Wo2),
                            ("c", Wqkvc, Woc)):
            Wq, Wk, Wv = Wqkv[:E], Wqkv[E:2 * E], Wqkv[2 * E:]
            m["wq" + l] = np.ascontiguousarray(Wq[hs].T)
            m["wk" + l] = np.ascontiguousarray(Wk[hs].T)
            m["wv" + l] = np.ascontiguousarray(Wv[hs].T)
            m["wo" + l] = np.ascontiguousarray(Wo[:, hs].T)
        m["wf1"] = np.ascontiguousarray(Wf1[fs].T)
        m["wf2"] = np.ascontiguousarray(Wf2[:, fs].T)
        in_maps.append(m)
    return in_maps


def run_device(in_maps, ncat_real, nrem_real, debug=False, trace=False):
    from concourse.bass_utils import run_bass_kernel_spmd

    key = (ncat_real, nrem_real, debug)
    if key not in _CACHE:
        _CACHE[key] = _build_device(ncat_real, nrem_real, debug=debug)
    nc = _CACHE[key]
    return run_bass_kernel_spmd(nc, in_maps, list(range(NCORES)), trace=trace)


def _kernel_impl(inputs, debug=False, trace=False):
    vision_feature = np.asarray(inputs["vision_feature"], np.float32)
    text_embed = np.asarray(inputs["text_embed"], np.float32)
    attention_mask = np.asarray(inputs["attention_mask"])

    biases_zero = all(
        not np.any(np.asarray(inputs[b]))
        for b in ("bqkv1", "bo1", "bqkv2", "bo2", "bqkvc", "boc",
                  "bf1", "bf2", "bs"))
    if (not bool(attention_mask.all())) or (not biases_zero):
        return _reference_np(**{k: np.asarray(v) for k, v in inputs.items()}), None

    t, sel_idx, rem_idx = _score_partition(vision_feature, text_embed,
                                           attention_mask)
    ncat_real = t + text_embed.shape[0]
    nrem_real = vision_feature.shape[0] - t
    kk = int(t * EXPAND)

    in_maps = _prep_in_maps(
        vision_feature, text_embed, sel_idx, rem_idx,
        _pad128(ncat_real), _pad128(nrem_real),
        np.asarray(inputs["Wqkv1"], np.float32),
        np.asarray(inputs["Wo1"], np.float32),
        np.asarray(inputs["Wqkv2"], np.float32),
        np.asarray(inputs["Wo2"], np.float32),
        np.asarray(inputs["Wqkvc"], np.float32),
        np.asarray(inputs["Woc"], np.float32),
        np.asarray(inputs["Wf1"], np.float32),
        np.asarray(inputs["Wf2"], np.float32),
        np.asarray(inputs["Ws"], np.float32))
    res = run_device(in_maps, ncat_real, nrem_real, debug=debug, trace=trace)
    logits = res.results[0]["logits"][0, :nrem_real]
    es = (1.0 / (1.0 + np.exp(-logits.astype(np.float32))))
    ei = np.argsort(-es, kind="stable")[:kk]
    final = np.sort(np.concatenate([sel_idx, rem_idx[ei]]))
    return vision_feature[final], res


def kernel(**inputs):
    out, _ = _kernel_impl(inputs)
    return out


# revision 15
# speedup vs baseline: 1.2022x; 1.0495x over previous
"""Trainium2 Bass kernel for nn_CosSimRouter_learn_49778670960796.

Host: cosine-similarity scoring / sort / gather (tiny, shape-determining).
Device (8 NeuronCores, tensor-parallel over heads/hidden):
  3x MHA + FFN + logits; fp32 storage, float32r matmuls; AllReduce after
  out-proj / FFN2 (Megatron-style TP). Activations feature-major [E, L].
Host: top-k + final gather (exact rows of the input).
"""

import numpy as np

E = 4096
H = 16
HID = 8192
GAMMA = 0.2
TEMP = 0.05
EXPAND = 0.7
NCORES = 8
ET = E // 128  # 32 feature tiles
DH = E // H  # 256
HL = H // NCORES  # 2 heads per core
DLOC = HL * DH  # 512 local head dims
FLOC = HID // NCORES  # 1024 local ffn hidden

_CACHE = {}


# ----------------------------------------------------------------------------
# host-side reference math (numpy, fp32) for the scoring stage + fallback
# ----------------------------------------------------------------------------

def _score_partition(vision_feature, text_embed, attention_mask):
    vf = vision_feature.astype(np.float32)
    te = text_embed.astype(np.float32)
    vn = vf / np.maximum(np.linalg.norm(vf, axis=-1, keepdims=True), 1e-8)
    tn = te / np.maximum(np.linalg.norm(te, axis=-1, keepdims=True), 1e-8)
    cs = vn @ tn.T
    cs = np.where(attention_mask[None, :], cs, np.float32(0.0))
    m = cs.max(axis=-1) / np.float32(TEMP)
    e = np.exp(m - m.max())
    scores = e / e.sum()
    order = np.argsort(-scores, kind="stable")
    cum = np.cumsum(scores[order])
    t = int((cum <= GAMMA).sum())
    return t, order[:t], order[t:]


def _ln_np(x):
    m = x.mean(-1, keepdims=True)
    v = ((x - m) ** 2).mean(-1, keepdims=True)
    return (x - m) / np.sqrt(v + 1e-5)


def _gelu_np(x):
    import math

    erf = np.frompyfunc(math.erf, 1, 1)
    return (x * 0.5 * (1.0 + erf(x / math.sqrt(2.0)).astype(np.float64))
            ).astype(x.dtype)


def _mha_np(q_in, kv_in, Wqkv, bqkv, Wo, bo):
    dh = E // H
    Wq, Wk, Wv = np.split(Wqkv, 3, axis=0)
    bq, bk, bv = np.split(bqkv, 3)
    q = (q_in @ Wq.T + bq).reshape(-1, H, dh)
    k = (kv_in @ Wk.T + bk).reshape(-1, H, dh)
    v = (kv_in @ Wv.T + bv).reshape(-1, H, dh)
    att = np.einsum("qhd,khd->hqk", q, k) / np.float32(np.sqrt(dh))
    att = att - att.max(-1, keepdims=True)
    att = np.exp(att)
    att /= att.sum(-1, keepdims=True)
    o = np.einsum("hqk,khd->qhd", att.astype(np.float32), v).reshape(-1, E)
    return o @ Wo.T + bo


def _reference_np(vision_feature, text_embed, attention_mask,
                  Wqkv1, bqkv1, Wo1, bo1, Wqkv2, bqkv2, Wo2, bo2,
                  Wqkvc, bqkvc, Woc, boc, Wf1, bf1, Wf2, bf2, Ws, bs):
    t, sel_idx, rem_idx = _score_partition(vision_feature, text_embed,
                                           attention_mask)
    sel = vision_feature[sel_idx]
    rem = vision_feature[rem_idx]
    cat = np.concatenate([sel, text_embed], axis=0)
    x = _ln_np(_mha_np(cat, cat, Wqkv1, bqkv1, Wo1, bo1) + cat)
    r = _ln_np(_mha_np(rem, rem, Wqkv2, bqkv2, Wo2, bo2) + rem)
    x = _ln_np(_mha_np(r, x, Wqkvc, bqkvc, Woc, boc) + r)
    ffn = _gelu_np(x @ Wf1.T + bf1) @ Wf2.T + bf2
    x = _ln_np(x + ffn)
    logits = (x @ Ws.T + bs).squeeze(-1)
    es = 1.0 / (1.0 + np.exp(-logits))
    k = int(t * EXPAND)
    ei = np.argsort(-es, kind="stable")[:k]
    final = np.sort(np.concatenate([sel_idx, rem_idx[ei]]))
    return vision_feature[final]


# ----------------------------------------------------------------------------
# device program
# ----------------------------------------------------------------------------

def _pad128(n):
    return ((n + 127) // 128) * 128


def _build_device(ncat_real, nrem_real, debug=False):
    import concourse.bacc as bacc
    import concourse.mybir as mybir
    import concourse.tile as tile

    dt = mybir.dt
    F32 = dt.float32
    F32R = dt.float32r
    BF16 = dt.bfloat16
    AF = mybir.ActivationFunctionType
    ALU = mybir.AluOpType

    ncat = _pad128(ncat_real)
    nrem = _pad128(nrem_real)
    JC = ncat // 128  # kv tiles for cat (2)
    JR = nrem // 128  # kv tiles for rem (4)

    nc = bacc.Bacc("TRN2", target_bir_lowering=False, debug=False,
                   num_devices=NCORES)

    # ---------------- DRAM I/O ----------------
    catT_d = nc.dram_tensor("catT", [E, ncat], F32R, kind="ExternalInput")
    remT_d = nc.dram_tensor("remT", [E, nrem], F32R, kind="ExternalInput")
    wd = {}
    for l in ("1", "2", "c"):
        for p in ("q", "k", "v"):
            wd[p + l] = nc.dram_tensor(f"w{p}{l}", [E, DLOC], F32R,
                                       kind="ExternalInput")
        wd["o" + l] = nc.dram_tensor(f"wo{l}", [DLOC, E], F32R,
                                     kind="ExternalInput")
    wd["f1"] = nc.dram_tensor("wf1", [E, FLOC], F32R, kind="ExternalInput")
    wd["f2"] = nc.dram_tensor("wf2", [FLOC, E], F32R, kind="ExternalInput")
    wd["s"] = nc.dram_tensor("ws", [128, ET], F32R, kind="ExternalInput")
    wsb_d = nc.dram_tensor("wsb", [128, E // NCORES // 128], F32R,
                           kind="ExternalInput")
    masks_d = nc.dram_tensor("masks", [128, 4], F32R, kind="ExternalInput")
    logits_d = nc.dram_tensor("logits", [1, nrem], F32, kind="ExternalOutput")
    dbg = {}
    if debug:
        for nm, L in (("dbg_x1", ncat), ("dbg_r", nrem), ("dbg_x2", nrem),
                      ("dbg_x3", nrem)):
            dbg[nm] = nc.dram_tensor(nm, [E, L], F32, kind="ExternalOutput")

    replica = [list(range(NCORES))]

    with tile.TileContext(nc, num_cores=NCORES) as tc:
        with (
            tc.tile_pool(name="acts", bufs=1) as acts,
            tc.tile_pool(name="psum", bufs=1, space="PSUM") as psum,
            tc.tile_pool(name="dram", bufs=1, space="DRAM") as dram,
        ):
            # ---- constants / packed stat tiles ----
            ones_col = acts.tile([128, 1], F32R, name="ones_col",
                                 tag="ones_col")
            nc.vector.memset(ones_col[:].bitcast(F32), 1.0)
            ones_row = acts.tile([1, 128], F32R, name="ones_row",
                                 tag="ones_row")
            nc.vector.memset(ones_row[:].bitcast(F32), 1.0)
            masks = acts.tile([128, 4], F32R, name="masks", tag="masks")
            nc.sync.dma_start(masks[:], masks_d.ap())

            def pp(name, L):
                return psum.tile([128, L], F32, name=name, tag="pp", bufs=8)

            def pstat(name, L):
                return psum.tile([1, L], F32, name=name, tag="pp", bufs=8)

            def wtile(name, cols):
                return acts.tile([128, cols], F32R, name=name, tag="wt",
                                 bufs=6, padded_shape=[128, 1024])

            # ---------------- building blocks ----------------
            def load_xT(name, dram_t, L, tagbase):
                ts = []
                for k in range(ET):
                    xt = acts.tile([128, L], F32R, name=f"{name}_{k}",
                                   tag=f"{tagbase}_{k}")
                    nc.sync.dma_start(xt[:],
                                      dram_t.ap()[128 * k:128 * (k + 1), :])
                    ts.append(xt)
                return ts

            def proj_fm(tagbase, w_dram, x_tiles, L, outtag):
                """q/k fm projection -> 4 tiles [128, L] (f32r)."""
                ps = [pp(f"ps_{tagbase}_{m}", L) for m in range(4)]
                outs = []
                for k in range(ET):
                    wt = wtile(f"w_{tagbase}_{k}", DLOC)
                    nc.sync.dma_start(
                        wt[:], w_dram.ap()[128 * k:128 * (k + 1), :])
                    for m in range(4):
                        nc.tensor.matmul(ps[m][:],
                                         wt[:, 128 * m:128 * (m + 1)],
                                         x_tiles[k][:],
                                         start=(k == 0), stop=(k == ET - 1))
                for m in range(4):
                    o = acts.tile([128, L], F32R, name=f"{tagbase}_{m}",
                                  tag=f"{outtag}_{m}")
                    nc.vector.tensor_copy(o[:], ps[m][:])
                    outs.append(o)
                return outs

            def proj_tm(tagbase, w_dram, x_tiles, L):
                """v tm projection -> L//128 tiles [128, DLOC] (f32r)."""
                jt = L // 128
                ps = [pp(f"ps_{tagbase}_{j}", DLOC) for j in range(jt)]
                outs = []
                for k in range(ET):
                    wt = wtile(f"w_{tagbase}_{k}", DLOC)
                    nc.sync.dma_start(
                        wt[:], w_dram.ap()[128 * k:128 * (k + 1), :])
                    for j in range(jt):
                        nc.tensor.matmul(ps[j][:],
                                         x_tiles[k][:, 128 * j:128 * (j + 1)],
                                         wt[:],
                                         start=(k == 0), stop=(k == ET - 1))
                for j in range(jt):
                    o = acts.tile([128, DLOC], F32R, name=f"{tagbase}_{j}",
                                  tag=f"v_{j}")
                    nc.vector.tensor_copy(o[:], ps[j][:])
                    outs.append(o)
                return outs

            def attention(tag, qT, kT, vT, Lq, Lkv, kv_valid, mask_idx):
                jt = Lkv // 128
                oT = []
                for h in range(HL):
                    exps = []
                    for j in range(jt):
                        p = pp(f"ps_s_{tag}_{h}_{j}", Lq)
                        for c in range(2):
                            nc.tensor.matmul(
                                p[:],
                                kT[2 * h + c][:, 128 * j:128 * (j + 1)],
                                qT[2 * h + c][:],
                                start=(c == 0), stop=(c == 1))
                        e = acts.tile([128, Lq], F32R,
                                      name=f"es_{tag}_{h}_{j}",
                                      tag=f"expS_{j}")
                        nc.scalar.activation(e[:], p[:], AF.Exp,
                                             scale=float(1.0 / np.sqrt(DH)))
                        exps.append(e)
                    dsum = pstat(f"ps_d_{tag}_{h}", Lq)
                    for j in range(jt):
                        if j == jt - 1 and kv_valid < Lkv:
                            col = masks[:, mask_idx:mask_idx + 1]
                        else:
                            col = ones_col[:]
                        nc.tensor.matmul(dsum[:], col, exps[j][:],
                                         start=(j == 0), stop=(j == jt - 1))
                    den = acts.tile([1, Lq], F32, name=f"den_{tag}_{h}",
                                    tag="aden")
                    rec = acts.tile([1, Lq], F32, name=f"rec_{tag}_{h}",
                                    tag="arec")
                    nc.vector.tensor_copy(den[:], dsum[:])
                    nc.vector.reciprocal(rec[:], den[:])
                    nc.vector.tensor_tensor(den[:], den[:], rec[:], ALU.mult)
                    nc.vector.tensor_scalar(den[:], den[:], -1.0, 2.0,
                                            ALU.mult, ALU.add)
                    rec2 = acts.tile([1, Lq], F32R, name=f"rec2_{tag}_{h}",
                                     tag="rec2")
                    nc.vector.tensor_tensor(rec2[:], rec[:], den[:], ALU.mult)
                    rrep_p = pp(f"ps_rr_{tag}_{h}", Lq)
                    nc.tensor.matmul(rrep_p[:], ones_row[:], rec2[:],
                                     start=True, stop=True)
                    rrep = acts.tile([128, Lq], F32, name=f"rr_{tag}_{h}",
                                     tag="rrep")
                    nc.scalar.copy(rrep[:], rrep_p[:])
                    for c in range(2):
                        po = pp(f"ps_o_{tag}_{h}_{c}", Lq)
                        for j in range(jt):
                            nc.tensor.matmul(
                                po[:],
                                vT[j][:, 256 * h + 128 * c:
                                      256 * h + 128 * (c + 1)],
                                exps[j][:],
                                start=(j == 0), stop=(j == jt - 1))
                        o = acts.tile([128, Lq], F32R,
                                      name=f"oT_{tag}_{h}_{c}",
                                      tag=f"oT_{2 * h + c}")
                        nc.vector.tensor_tensor(o[:], po[:], rrep[:],
                                                ALU.mult)
                        oT.append(o)
                return oT

            def out_proj_to_dram(tag, oT, w_dram, ar_in, Lq, sdt):
                for quarter in range(4):
                    wo_t = []
                    for k in range(4):
                        wt = wtile(f"wo_{tag}_{quarter}_{k}", 1024)
                        nc.sync.dma_start(
                            wt[:],
                            w_dram.ap()[128 * k:128 * (k + 1),
                                        1024 * quarter:1024 * (quarter + 1)])
                        wo_t.append(wt)
                    ps = []
                    for mm in range(8):
                        m = 8 * quarter + mm
                        ps.append(pp(f"ps_op_{tag}_{m}", Lq))
                    for k in range(4):
                        for mm in range(8):
                            nc.tensor.matmul(
                                ps[mm][:],
                                wo_t[k][:, 128 * mm:128 * (mm + 1)],
                                oT[k][:],
                                start=(k == 0), stop=(k == 3))
                    for mm in range(8):
                        m = 8 * quarter + mm
                        st = acts.tile([128, Lq], sdt,
                                       name=f"st_{tag}_{m}", tag="stage",
                                       bufs=3)
                        nc.vector.tensor_copy(st[:], ps[mm][:])
                        nc.sync.dma_start(
                            ar_in[128 * m:128 * (m + 1), :], st[:])

            def do_allreduce(tag, ar_in, ar_out):
                nc.gpsimd.collective_compute(
                    "AllReduce", ALU.add, replica_groups=replica,
                    ins=[ar_in.opt()], outs=[ar_out.opt()])

            def residual_ln(tag, ar_out, res_tiles, L, adt=F32, valid=None,
                            dump=None, normalize=True):
                """In-place: res_tiles[k] <- LN(ar_out + res_tiles)[k]."""
                # xsum (in-place into res slot)
                for k in range(ET):
                    b = acts.tile([128, L], adt, name=f"arb_{tag}_{k}",
                                  tag="arb", bufs=4)
                    nc.sync.dma_start(b[:], ar_out[128 * k:128 * (k + 1), :])
                    nc.vector.tensor_tensor(res_tiles[k][:], b[:],
                                            res_tiles[k][:], ALU.add)
                s1p = pstat(f"ps_s1_{tag}", L)
                s2p = pstat(f"ps_s2_{tag}", L)
                for k in range(ET):
                    nc.tensor.matmul(s1p[:], ones_col[:], res_tiles[k][:],
                                     start=(k == 0), stop=(k == ET - 1))
                for k in range(ET):
                    sq = acts.tile([128, L], F32R, name=f"sq_{tag}_{k}",
                                   tag="stage", bufs=3)
                    nc.scalar.square(sq[:], res_tiles[k][:])
                    nc.tensor.matmul(s2p[:], ones_col[:], sq[:],
                                     start=(k == 0), stop=(k == ET - 1))
                mean = acts.tile([1, L], F32, name=f"mean_{tag}",
                                 tag="lmean")
                var = acts.tile([1, L], F32, name=f"var_{tag}", tag="lvar")
                tmpa = acts.tile([1, L], F32, name=f"tmpa_{tag}", tag="ltmp")
                r0 = acts.tile([1, L], F32, name=f"r0_{tag}", tag="lr0")
                nc.scalar.mul(mean[:], s1p[:], 1.0 / E)
                nc.scalar.mul(var[:], s2p[:], 1.0 / E)
                nc.scalar.square(tmpa[:], mean[:])
                nc.vector.tensor_sub(var[:], var[:], tmpa[:])
                nc.vector.tensor_scalar_add(var[:], var[:], 1e-5)
                nc.scalar.sqrt(tmpa[:], var[:])
                nc.vector.reciprocal(r0[:], tmpa[:])
                nc.vector.tensor_tensor(tmpa[:], r0[:], r0[:], ALU.mult)
                nc.vector.tensor_tensor(tmpa[:], tmpa[:], var[:], ALU.mult)
                nc.vector.tensor_scalar(tmpa[:], tmpa[:], -0.5, 1.5, ALU.mult,
                                        ALU.add)
                rstd = acts.tile([1, L], F32R, name=f"rstd_{tag}", tag="rstd")
                nmr = acts.tile([1, L], F32R, name=f"nmr_{tag}", tag="nmr")
                nc.vector.tensor_tensor(rstd[:], r0[:], tmpa[:], ALU.mult)
                nc.vector.scalar_tensor_tensor(nmr[:], mean[:], -1.0, rstd[:],
                                               ALU.mult, ALU.mult)
                if not normalize:
                    return rstd, nmr
                Apsum = pp(f"ps_A_{tag}", L)
                nc.tensor.matmul(Apsum[:], ones_row[:], rstd[:], start=True,
                                 stop=True)
                Bpsum = pp(f"ps_B_{tag}", L)
                nc.tensor.matmul(Bpsum[:], ones_row[:], nmr[:], start=True,
                                 stop=True)
                Asb = acts.tile([128, L], F32, name=f"A_{tag}", tag="Asb")
                nc.scalar.copy(Asb[:], Apsum[:])
                Bsb = acts.tile([128, L], F32, name=f"B_{tag}", tag="Bsb")
                nc.scalar.copy(Bsb[:], Bpsum[:])
                for k in range(ET):
                    nc.vector.tensor_tensor(res_tiles[k][:], res_tiles[k][:],
                                            Asb[:], ALU.mult)
                    nc.vector.tensor_tensor(res_tiles[k][:], res_tiles[k][:],
                                            Bsb[:], ALU.add)
                    if valid is not None and valid < L:
                        nc.vector.memset(
                            res_tiles[k][:, valid:L].bitcast(F32), 0.0)
                    if dump is not None:
                        nc.sync.dma_start(
                            dump.ap()[128 * k:128 * (k + 1), :],
                            res_tiles[k][:].bitcast(F32))
                return res_tiles

            # ================= program =================
            # fm512 family ("a_{k}"): remT -> r -> x2 -> x3 (in-place chain)
            # fm256 family ("b_{k}"): catT -> x1
            a_t = load_xT("remT", remT_d, nrem, "a")

            # ---- MHA2 (rem self-attention) ----
            q2 = proj_fm("q2", wd["q2"], a_t, nrem, "q")
            k2 = proj_fm("k2", wd["k2"], a_t, nrem, "k")
            v2 = proj_tm("v2", wd["v2"], a_t, nrem)
            o2 = attention("a2", q2, k2, v2, nrem, nrem, nrem_real, 1)
            arin2 = dram.tile([E, nrem], BF16, name="arin2", tag="arin2")
            arout2 = dram.tile([E, nrem], BF16, name="arout2", tag="arout2", addr_space="Shared")
            out_proj_to_dram("op2", o2, wd["o2"], arin2, nrem, BF16)
            do_allreduce("2", arin2, arout2)

            # ---- MHA1 (cat self-attention), overlaps AR2 ----
            b_t = load_xT("catT", catT_d, ncat, "b")
            q1 = proj_fm("q1", wd["q1"], b_t, ncat, "q")
            k1 = proj_fm("k1", wd["k1"], b_t, ncat, "k")
            v1 = proj_tm("v1", wd["v1"], b_t, ncat)
            o1 = attention("a1", q1, k1, v1, ncat, ncat, ncat_real, 0)
            arin1 = dram.tile([E, ncat], BF16, name="arin1", tag="arin1")
            arout1 = dram.tile([E, ncat], BF16, name="arout1", tag="arout1", addr_space="Shared")
            out_proj_to_dram("op1", o1, wd["o1"], arin1, ncat, BF16)
            do_allreduce("1", arin1, arout1)

            # ---- LN stages: r = LN(AR2 + rem); x1 = LN(AR1 + cat) ----
            r_t = residual_ln("r", arout2, a_t, nrem, adt=BF16,
                              dump=dbg.get("dbg_r"))
            x1_t = residual_ln("x1", arout1, b_t, ncat, adt=BF16,
                               valid=ncat_real, dump=dbg.get("dbg_x1"))

            # ---- MHAc (q from r, kv from x1) ----
            qc = proj_fm("qc", wd["qc"], r_t, nrem, "q")
            kc = proj_fm("kc", wd["kc"], x1_t, ncat, "k")
            vc = proj_tm("vc", wd["vc"], x1_t, ncat)
            oc = attention("ac", qc, kc, vc, nrem, ncat, ncat_real, 0)
            arinc = dram.tile([E, nrem], BF16, name="arinc", tag="arinc")
            aroutc = dram.tile([E, nrem], BF16, name="aroutc", tag="aroutc", addr_space="Shared")
            out_proj_to_dram("opc", oc, wd["oc"], arinc, nrem, BF16)
            do_allreduce("c", arinc, aroutc)
            x2_t = residual_ln("x2", aroutc, r_t, nrem, adt=BF16,
                               dump=dbg.get("dbg_x2"))

            # ---- FFN ----
            # f1: hT = gelu(Wf1_shard @ x2): 8 psums, single weight sweep
            ps_f1 = [pp(f"ps_f1_{m}", nrem) for m in range(8)]
            for k in range(ET):
                wt = wtile(f"w_f1_{k}", FLOC)
                nc.sync.dma_start(
                    wt[:], wd["f1"].ap()[128 * k:128 * (k + 1), :])
                for m in range(8):
                    nc.tensor.matmul(ps_f1[m][:],
                                     wt[:, 128 * m:128 * (m + 1)],
                                     x2_t[k][:],
                                     start=(k == 0), stop=(k == ET - 1))
            hT = []
            for m in range(8):
                tg = f"v_{m}" if m < 4 else f"q_{m - 4}"
                h = acts.tile([128, nrem], F32R, name=f"hT_{m}", tag=tg)
                nc.scalar.activation(h[:], ps_f1[m][:], AF.Gelu)
                hT.append(h)
            # f2: quarters of output cols; psum group of 8 m-tiles per quarter
            arin4 = dram.tile([E, nrem], F32, name="arin4", tag="arin4")
            arout4 = dram.tile([E, nrem], F32, name="arout4", tag="arout4", addr_space="Shared")
            HK = FLOC // 128  # 8
            for quarter in range(4):
                ps = []
                for mm in range(8):
                    m = 8 * quarter + mm
                    ps.append(pp(f"ps_f2_{m}", nrem))
                for khalf in range(2):
                    wf_t = []
                    for kk in range(4):
                        k = 4 * khalf + kk
                        wt = wtile(f"w_f2_{quarter}_{k}", 1024)
                        nc.sync.dma_start(
                            wt[:],
                            wd["f2"].ap()[128 * k:128 * (k + 1),
                                          1024 * quarter:1024 * (quarter + 1)])
                        wf_t.append(wt)
                    for kk in range(4):
                        k = 4 * khalf + kk
                        for mm in range(8):
                            nc.tensor.matmul(
                                ps[mm][:],
                                wf_t[kk][:, 128 * mm:128 * (mm + 1)],
                                hT[k][:],
                                start=(k == 0), stop=(k == HK - 1))
                for mm in range(8):
                    m = 8 * quarter + mm
                    st = acts.tile([128, nrem], F32, name=f"st_f2_{m}",
                                   tag="stage", bufs=3)
                    if debug:
                        nc.vector.tensor_copy(st[:], ps[mm][:])
                    else:
                        # fold residual: st = x2/NCORES + partial, so the
                        # cross-core sum of st equals x2 + ffn
                        nc.vector.scalar_tensor_tensor(
                            st[:], x2_t[m][:], 1.0 / NCORES, ps[mm][:],
                            ALU.mult, ALU.add)
                    nc.sync.dma_start(arin4[128 * m:128 * (m + 1), :], st[:])
            if debug:
                do_allreduce("4", arin4, arout4)
                rstd3, nmr3 = residual_ln("x3", arout4, x2_t, nrem,
                                          normalize=True,
                                          dump=dbg.get("dbg_x3"))
                ws_sb = acts.tile([128, ET], F32R, name="ws_sb", tag="ws_sb")
                nc.sync.dma_start(ws_sb[:], wd["s"].ap())
                lp = pstat("ps_logit", nrem)
                for k in range(ET):
                    nc.tensor.matmul(lp[:], ws_sb[:, k:k + 1], x2_t[k][:],
                                     start=(k == 0), stop=(k == ET - 1))
                lsb = acts.tile([1, nrem], F32, name="lsb", tag="lsb")
                nc.vector.tensor_copy(lsb[:], lp[:])
                nc.sync.dma_start(logits_d.ap(), lsb[:])
            else:
                # ReduceScatter xsum over feature blocks; local partial
                # stats; tiny AllReduce of [s1, s2, wsdot]; logits via the
                # affine-LN identity.
                EB = E // NCORES  # 512 features per core
                rs4 = dram.tile([EB, nrem], F32, name="rs4", tag="rs4")
                nc.gpsimd.collective_compute(
                    "ReduceScatter", ALU.add, replica_groups=replica,
                    ins=[arin4.opt()], outs=[rs4.opt()])
                wsb_sb = acts.tile([128, EB // 128], F32R, name="wsb_sb",
                                   tag="ws_sb")
                nc.sync.dma_start(wsb_sb[:], wsb_d.ap())
                s1p = pstat("ps_rs1", nrem)
                s2p = pstat("ps_rs2", nrem)
                wsp = pstat("ps_rsw", nrem)
                bts = []
                for k in range(EB // 128):
                    bt = acts.tile([128, nrem], F32R, name=f"rsb_{k}",
                                   tag="arb", bufs=4)
                    nc.gpsimd.dma_start(bt[:], rs4[128 * k:128 * (k + 1), :])
                    bts.append(bt)
                for k in range(EB // 128):
                    nc.tensor.matmul(s1p[:], ones_col[:], bts[k][:],
                                     start=(k == 0), stop=(k == 3))
                    nc.tensor.matmul(wsp[:], wsb_sb[:, k:k + 1], bts[k][:],
                                     start=(k == 0), stop=(k == 3))
                for k in range(EB // 128):
                    sq = acts.tile([128, nrem], F32R, name=f"rssq_{k}",
                                   tag="stage", bufs=3)
                    nc.scalar.square(sq[:], bts[k][:])
                    nc.tensor.matmul(s2p[:], ones_col[:], sq[:],
                                     start=(k == 0), stop=(k == 3))
                s1s = acts.tile([1, nrem], F32, name="s1s", tag="lmean")
                s2s = acts.tile([1, nrem], F32, name="s2s", tag="lvar")
                wss = acts.tile([1, nrem], F32, name="wss", tag="lr0")
                nc.vector.tensor_copy(s1s[:], s1p[:])
                nc.vector.tensor_copy(s2s[:], s2p[:])
                nc.vector.tensor_copy(wss[:], wsp[:])
                arin5 = dram.tile([4, nrem], F32, name="arin5", tag="arin5")
                arout5 = dram.tile([4, nrem], F32, name="arout5",
                                   tag="arout5", addr_space="Shared")
                nc.sync.dma_start(arin5[0:1, :], s1s[:])
                nc.sync.dma_start(arin5[1:2, :], s2s[:])
                nc.sync.dma_start(arin5[2:3, :], wss[:])
                nc.sync.dma_start(arin5[3:4, :], s1s[:])
                nc.gpsimd.collective_compute(
                    "AllReduce", ALU.add, replica_groups=replica,
                    ins=[arin5.opt()], outs=[arout5.opt()])
                g1 = acts.tile([1, nrem], F32, name="g1", tag="aden")
                g2 = acts.tile([1, nrem], F32, name="g2", tag="arec")
                g3 = acts.tile([1, nrem], F32, name="g3", tag="wsd")
                nc.sync.dma_start(g1[:], arout5[0:1, :])
                nc.sync.dma_start(g2[:], arout5[1:2, :])
                nc.sync.dma_start(g3[:], arout5[2:3, :])
                mean = acts.tile([1, nrem], F32, name="mean_l", tag="lmean")
                var = acts.tile([1, nrem], F32, name="var_l", tag="lvar")
                tmpa = acts.tile([1, nrem], F32, name="tmpa_l", tag="ltmp")
                r0 = acts.tile([1, nrem], F32, name="r0_l", tag="lr0")
                nc.scalar.mul(mean[:], g1[:], 1.0 / E)
                nc.scalar.mul(var[:], g2[:], 1.0 / E)
                nc.scalar.square(tmpa[:], mean[:])
                nc.vector.tensor_sub(var[:], var[:], tmpa[:])
                nc.vector.tensor_scalar_add(var[:], var[:], 1e-5)
                nc.scalar.sqrt(tmpa[:], var[:])
                nc.vector.reciprocal(r0[:], tmpa[:])
                nc.vector.tensor_tensor(tmpa[:], r0[:], r0[:], ALU.mult)
                nc.vector.tensor_tensor(tmpa[:], tmpa[:], var[:], ALU.mult)
                nc.vector.tensor_scalar(tmpa[:], tmpa[:], -0.5, 1.5,
                                        ALU.mult, ALU.add)
                rstd = acts.tile([1, nrem], F32, name="rstd_l", tag="rstd")
                nc.vector.tensor_tensor(rstd[:], r0[:], tmpa[:], ALU.mult)
                nmr = acts.tile([1, nrem], F32, name="nmr_l", tag="nmr")
                nc.vector.scalar_tensor_tensor(nmr[:], mean[:], -1.0,
                                               rstd[:], ALU.mult, ALU.mult)
                wdot = acts.tile([1, nrem], F32, name="wdot", tag="wdot")
                nc.vector.tensor_tensor(wdot[:], rstd[:], g3[:], ALU.mult)
                lsb = acts.tile([1, nrem], F32, name="lsb", tag="lsb")
                nc.vector.scalar_tensor_tensor(lsb[:], nmr[:],
                                               masks[0:1, 2:3], wdot[:],
                                               ALU.mult, ALU.add)
                nc.sync.dma_start(logits_d.ap(), lsb[:])

    nc.compile()
    return nc


# ----------------------------------------------------------------------------
# host orchestration
# ----------------------------------------------------------------------------

def _prep_in_maps(vision_feature, text_embed, sel_idx, rem_idx, ncat, nrem,
                  Wqkv1, Wo1, Wqkv2, Wo2, Wqkvc, Woc, Wf1, Wf2, Ws):
    f32 = np.float32
    sel = vision_feature[sel_idx]
    rem = vision_feature[rem_idx]
    cat = np.concatenate([sel, text_embed], axis=0)
    catT = np.zeros((E, ncat), f32)
    catT[:, :cat.shape[0]] = cat.T
    remT = np.zeros((E, nrem), f32)
    remT[:, :rem.shape[0]] = rem.T

    ncat_real = cat.shape[0]
    nrem_real = rem.shape[0]
    masks = np.zeros((128, 4), f32)
    masks[:ncat_real - 128 * (ncat // 128 - 1), 0] = 1.0
    masks[:nrem_real - 128 * (nrem // 128 - 1), 1] = 1.0
    masks[0, 2] = Ws.astype(np.float64).sum()

    in_maps = []
    for c in range(NCORES):
        hs = slice(DLOC * c, DLOC * (c + 1))
        fs = slice(FLOC * c, FLOC * (c + 1))
        eb = E // NCORES
        m = {"catT": catT, "remT": remT, "masks": masks,
             "ws": np.ascontiguousarray(Ws[0].reshape(ET, 128).T),
             "wsb": np.ascontiguousarray(
                 Ws[0, eb * c:eb * (c + 1)].reshape(eb // 128, 128).T)}
        for l, Wqkv, Wo in (("1", Wqkv1, Wo1), ("2", Wqkv2, Wo2),
                            ("c", Wqkvc, Woc)):
            Wq, Wk, Wv = Wqkv[:E], Wqkv[E:2 * E], Wqkv[2 * E:]
            m["wq" + l] = np.ascontiguousarray(Wq[hs].T)
            m["wk" + l] = np.ascontiguousarray(Wk[hs].T)
            m["wv" + l] = np.ascontiguousarray(Wv[hs].T)
            m["wo" + l] = np.ascontiguousarray(Wo[:, hs].T)
        m["wf1"] = np.ascontiguousarray(Wf1[fs].T)
        m["wf2"] = np.ascontiguousarray(Wf2[:, fs].T)
        in_maps.append(m)
    return in_maps


def run_device(in_maps, ncat_real, nrem_real, debug=False, trace=False):
    from concourse.bass_utils import run_bass_kernel_spmd

    key = (ncat_real, nrem_real, debug)
    if key not in _CACHE:
        _CACHE[key] = _build_device(ncat_real, nrem_real, debug=debug)
    nc = _CACHE[key]
    return run_bass_kernel_spmd(nc, in_maps, list(range(NCORES)), trace=trace)


def _kernel_impl(inputs, debug=False, trace=False):
    vision_feature = np.asarray(inputs["vision_feature"], np.float32)
    text_embed = np.asarray(inputs["text_embed"], np.float32)
    attention_mask = np.asarray(inputs["attention_mask"])

    biases_zero = all(
        not np.any(np.asarray(inputs[b]))
        for b in ("bqkv1", "bo1", "bqkv2", "bo2", "bqkvc", "boc",
                  "bf1", "bf2", "bs"))
    if (not bool(attention_mask.all())) or (not biases_zero):
        return _reference_np(**{k: np.asarray(v) for k, v in inputs.items()}), None

    t, sel_idx, rem_idx = _score_partition(vision_feature, text_embed,
                                           attention_mask)
    ncat_real = t + text_embed.shape[0]
    nrem_real = vision_feature.shape[0] - t
    kk = int(t * EXPAND)

    in_maps = _prep_in_maps(
        vision_feature, text_embed, sel_idx, rem_idx,
        _pad128(ncat_real), _pad128(nrem_real),
        np.asarray(inputs["Wqkv1"], np.float32),
        np.asarray(inputs["Wo1"], np.float32),
        np.asarray(inputs["Wqkv2"], np.float32),
        np.asarray(inputs["Wo2"], np.float32),
        np.asarray(inputs["Wqkvc"], np.float32),
        np.asarray(inputs["Woc"], np.float32),
        np.asarray(inputs["Wf1"], np.float32),
        np.asarray(inputs["Wf2"], np.float32),
        np.asarray(inputs["Ws"], np.float32))
    res = run_device(in_maps, ncat_real, nrem_real, debug=debug, trace=trace)
    logits = res.results[0]["logits"][0, :nrem_real]
    es = (1.0 / (1.0 + np.exp(-logits.astype(np.float32))))
    ei = np.argsort(-es, kind="stable")[:kk]
    final = np.sort(np.concatenate([sel_idx, rem_idx[ei]]))
    return vision_feature[final], res


def kernel(**inputs):
    out, _ = _kernel_impl(inputs)
    return out
